# revision 6
# baseline (speedup 1.0000x reference)
"""BiRNN language-model kernel for 8 Trainium2 NeuronCores.

Strategy: data-parallel over the batch dim (B=32 -> 4 per core), no
collectives.  Per core:
  1. indirect-DMA gather of the core's S*4 embedding rows (natural order
     for the L->R scan, time-reversed order for the R->L scan)
  2. per-128-token-chunk: PE transposes -> x-projection matmul
     xprojT[64, S*4] (rows 0:30 = W_e_lr^T emb, rows 32:62 = W_e_rl^T
     emb_rev), pre-injected (with input biases and a constant tanh(8)=1
     lane that later carries b_ho) into one PSUM bank via an identity
     matmul, then the scan steps for that chunk -- the whole pipeline
     streams behind the gather.
  3. sequential scan, both directions stacked: each step is ONE
     accumulating [64,64]@[64,4] fp16 matmul + tanh.
  4. output projection + log_softmax over V=32000, two chunk-pairs at a
     time using PE row-group packing (two concurrent K=64 matmuls in
     partitions 0:64 / 64:128 -- the PE clock is capped at 1.2 GHz here,
     packing doubles throughput).  pass1: logits -> PSUM, ACT exp with
     accum_out row sums; logZ = ln(Z) via a bit-trick + 3 exp-Newton
     iterations (avoids ACT table switches -- tanh/exp share one table
     set).  pass2: recompute logits, subtract logZ during the PSUM->SBUF
     copy, 1MB DMA stores.

fp32 matmuls run 4 cycles/row (LOW_HIGH dual pass) on trn2, so all
matmuls use fp16 operands with fp32 PSUM accumulation (measured
end-to-end rel err ~5e-5).  All SBUF access patterns must start at
partition 0/32/64/96, hence direction blocks are padded 30->32 rows;
pad rows multiply zeroed weight rows so they never contribute.
"""

import sys

import numpy as np

for _p in ("/opt/trn_rl_repo", "/root/.axon_site/_ro/trn_rl_repo"):
    if _p not in sys.path:
        sys.path.insert(0, _p)

# problem constants
S, B, V, E, H = 128, 32, 32000, 150, 30
NCORES = 8
BL = B // NCORES          # batch rows per core
HP = 32                   # H padded to the 32-partition alignment
DH = 2 * HP               # 64: stacked direction state rows (= matmul K)
LANE = 62                 # constant-one lane (carries b_ho), row 62 of 64
EH = 128                  # embedding dims handled by the "hi" K-split
EL = E - EH               # 22 remaining dims
VS = 512                  # fp32 matmul free-dim max (one PSUM bank)
SUP = 1024                # supertile: 2 PSUM banks (x2 chunk-halves x2 bufs)
LN2 = float(np.log(2.0))

# packed "smalls16" column layout (fp16, [128, n]):
C_WLRH, C_WRLH, C_WLRL, C_WRLL = 0, 30, 60, 90
C_WHB, C_I64B, C_INIT = 120, 184, 248
# packed "smalls32": just the 128x128 identity
C_IDENT = 0


def _v_supertiles(v_total):
    tiles = []
    v0 = 0
    while v0 < v_total:
        w = min(SUP, v_total - v0)
        tiles.append((v0, w))
        v0 += w
    return tiles


def _splits512(w):
    out = []
    k0 = 0
    while k0 < w:
        kw = min(VS, w - k0)
        out.append((k0, kw))
        k0 += kw
    return out


def _chunk_map(s, bl, nch):
    """chunk -> (half, window): order chunks by scan-readiness and pair
    the two earliest, next two, ...  half 0 lives in hcatP rows 0:64,
    half 1 in rows 64:128; window w is hcatP cols w*128:(w+1)*128."""
    tw = 128 // bl                    # tokens per chunk
    ready = lambda ch: max(tw * ch + tw - 2, s - 2 - tw * ch)
    order = sorted(range(nch), key=ready)
    cmap = {}
    for pos, ch in enumerate(order):
        cmap[ch] = (pos % 2, pos // 2)
    return cmap, [(order[2 * p], order[2 * p + 1]) for p in range(nch // 2)]


def build_program(s=S, bl=BL, v=V):
    """Build the per-core Bass program (identical on all cores)."""
    from concourse import bacc, mybir
    import concourse.tile as tile

    f32 = mybir.dt.float32
    f16 = mybir.dt.float16
    i32u = mybir.dt.int32
    Act = mybir.ActivationFunctionType

    r = s * bl                 # rows per core
    nch = r // 128             # 128-row chunks
    tw = 128 // bl             # tokens per chunk
    assert r % 256 == 0, "need an even number of 128-row chunks"
    sup_tiles = _v_supertiles(v)
    ns = len(sup_tiles)
    cmap, pairs = _chunk_map(s, bl, nch)
    c_init = C_INIT + bl

    nc = bacc.Bacc(None, target_bir_lowering=False)

    idx_d = nc.dram_tensor("idx", [128, 2 * nch], i32u, kind="ExternalInput")
    emb_d = nc.dram_tensor("emb", [V, E], f32, kind="ExternalInput")
    w_dup_d = nc.dram_tensor("w_dup", [128, v], f16, kind="ExternalInput")
    s16_d = nc.dram_tensor("smalls16", [128, c_init], f16, kind="ExternalInput")
    s32_d = nc.dram_tensor("smalls32", [128, 128], f32, kind="ExternalInput")
    out_d = nc.dram_tensor("out", [r, v], f32, kind="ExternalOutput")

    from concourse import bass

    with tile.TileContext(nc) as tc:
        with (
            tc.tile_pool(name="persist", bufs=1) as pp,
            tc.tile_pool(name="stage", bufs=3) as stp,
            tc.tile_pool(name="esc", bufs=2) as escp,
            tc.tile_pool(name="stat", bufs=4) as statp,
        ):
            # ---- input loads (idx first: the gather chain is the long pole)
            idx = pp.tile([128, 2 * nch], i32u)
            nc.sync.dma_start(idx[:], idx_d[:])
            s16 = pp.tile([128, c_init], f16)
            nc.sync.dma_start(s16[:], s16_d[:])
            s32 = pp.tile([128, 128], f32)
            nc.sync.dma_start(s32[:], s32_d[:])
            w_dup = pp.tile([128, v], f16)
            nc.sync.dma_start(w_dup[:], w_dup_d[:])

            ident = s32[:, C_IDENT : C_IDENT + 128]
            we_lr_hi = s16[:, C_WLRH : C_WLRH + H]
            we_rl_hi = s16[:, C_WRLH : C_WRLH + H]
            we_lr_lo = s16[0:EL, C_WLRL : C_WLRL + H]
            we_rl_lo = s16[0:EL, C_WRLL : C_WRLL + H]
            wh_blk = s16[0:DH, C_WHB : C_WHB + DH]
            i64b = s16[0 : DH + 1, C_I64B : C_I64B + DH]   # I64 + bias row
            init_sb = s16[0:DH, C_INIT : C_INIT + bl]

            # ---- gathers (all issued up front; chunks stream through) -----
            embg_lr = pp.tile([128, nch, E], f32)
            embg_rl = pp.tile([128, nch, E], f32)
            for j in range(nch):
                nc.gpsimd.indirect_dma_start(
                    out=embg_lr[:, j, :], out_offset=None, in_=emb_d[:],
                    in_offset=bass.IndirectOffsetOnAxis(ap=idx[:, j : j + 1], axis=0),
                )
                nc.gpsimd.indirect_dma_start(
                    out=embg_rl[:, j, :], out_offset=None, in_=emb_d[:],
                    in_offset=bass.IndirectOffsetOnAxis(
                        ap=idx[:, nch + j : nch + j + 1], axis=0
                    ),
                )

            embT_hi_lr = pp.tile([EH, r], f16)
            embT_hi_rl = pp.tile([EH, r], f16)
            embT_lo_lr = pp.tile([EL, r], f16)
            embT_lo_rl = pp.tile([EL, r], f16)

            xprojT = pp.tile([DH + 1, r], f16)   # row 64 = ones (bias inject)
            nc.vector.memset(xprojT[:], 0.0)
            nc.vector.memset(xprojT[DH : DH + 1, :], 1.0)
            hcatP = pp.tile([128, (nch // 2) * 128], f16)
            nc.vector.memset(hcatP[:], 0.0)

            # init states: hLR[0] -> chunk 0 col 0, hRL[s] -> chunk nch-1 col 127
            h0, w0 = cmap[0]
            nc.vector.tensor_copy(
                hcatP[h0 * 64 : h0 * 64 + HP, w0 * 128 : w0 * 128 + bl],
                init_sb[0:HP, :],
            )
            h1, w1 = cmap[nch - 1]
            nc.vector.tensor_copy(
                hcatP[h1 * 64 + HP : h1 * 64 + DH, w1 * 128 + 128 - bl : w1 * 128 + 128],
                init_sb[HP:DH, :],
            )

            # ---- chunk-pipelined: transpose -> xproj -> prefill -> scan ----
            with (
                tc.tile_pool(name="pre_psum", bufs=2, space="PSUM") as prepsum,
                tc.tile_pool(name="xp_psum", bufs=2, space="PSUM") as xpp,
                tc.tile_pool(name="scan_psum", bufs=1, space="PSUM") as scp,
                tc.tile_pool(name="scanh", bufs=2) as shp,
            ):
                pscan = scp.tile([DH, VS], f32)
                hprev = init_sb
                for ch in range(nch):
                    cs = slice(ch * 128, (ch + 1) * 128)
                    for embg, ehi, elo in (
                        (embg_lr, embT_hi_lr, embT_lo_lr),
                        (embg_rl, embT_hi_rl, embT_lo_rl),
                    ):
                        tp = prepsum.tile([128, 128], f32, tag="tp")
                        nc.tensor.transpose(tp[:], embg[:, ch, 0:EH], ident)
                        nc.vector.tensor_copy(ehi[:, cs], tp[:])
                        tp2 = prepsum.tile([128, 128], f32, tag="tp")
                        nc.tensor.transpose(tp2[0:EL, :], embg[:, ch, EH:E], ident)
                        nc.vector.tensor_copy(elo[:, cs], tp2[0:EL, :])
                    for row0, whi, wlo, ehi, elo in (
                        (0, we_lr_hi, we_lr_lo, embT_hi_lr, embT_lo_lr),
                        (HP, we_rl_hi, we_rl_lo, embT_hi_rl, embT_lo_rl),
                    ):
                        psx = xpp.tile([H, 128], f32, tag="xp")
                        nc.tensor.matmul(psx[:], whi, ehi[:, cs], start=True, stop=False)
                        nc.tensor.matmul(psx[:], wlo, elo[:, cs], start=False, stop=True)
                        nc.vector.tensor_copy(xprojT[row0 : row0 + H, cs], psx[:])
                    # prefill this chunk's pre-activations (+ biases, + 8.0 on
                    # the LANE row whose tanh == 1.0 exactly in fp16)
                    pc0 = ch * 128
                    pcw = min(128, (s - 1) * bl - pc0)
                    if pcw > 0:
                        nc.tensor.matmul(
                            pscan[:, pc0 : pc0 + pcw], i64b,
                            xprojT[:, pc0 : pc0 + pcw],
                            start=(ch == 0), stop=False, skip_group_check=True,
                        )
                    # scan steps whose xproj lives in this chunk
                    for t in range(ch * tw, min((ch + 1) * tw, s - 1)):
                        sl = slice(t * bl, (t + 1) * bl)
                        nc.tensor.matmul(
                            pscan[:, sl], wh_blk, hprev,
                            start=False, stop=(t == s - 2), skip_group_check=True,
                        )
                        hn = shp.tile([DH, bl], f16, tag="h")
                        nc.scalar.activation(hn[:], pscan[:, sl], Act.Tanh)
                        # hLR[t+1] -> chunk (t+1)//tw, col (t+1)%tw
                        dch = (t + 1) // tw
                        dh_, dw = cmap[dch]
                        dc = dw * 128 + ((t + 1) % tw) * bl
                        nc.vector.tensor_copy(
                            hcatP[dh_ * 64 : dh_ * 64 + HP, dc : dc + bl], hn[0:HP, :]
                        )
                        # hRL[i+1] with i = s-2-t -> chunk i//tw
                        i_ = s - 2 - t
                        dch = i_ // tw
                        dh_, dw = cmap[dch]
                        dc = dw * 128 + (i_ % tw) * bl
                        nc.vector.tensor_copy(
                            hcatP[dh_ * 64 + HP : dh_ * 64 + DH, dc : dc + bl],
                            hn[HP:DH, :],
                        )
                        hprev = hn[:]

            # ---- output projection + log_softmax ----------------------------
            with (
                tc.tile_pool(name="opsA", bufs=2, space="PSUM") as opA,
                tc.tile_pool(name="opsB", bufs=2, space="PSUM") as opB,
            ):
                for p, (chA, chB) in enumerate(pairs):
                    lhsA = hcatP[0:DH, p * 128 : (p + 1) * 128]
                    lhsB = hcatP[DH:128, p * 128 : (p + 1) * 128]
                    sumsA = statp.tile([128, ns], f32, tag="sumsA")
                    sumsB = statp.tile([128, ns], f32, tag="sumsB")
                    for sti, (v0, w) in enumerate(sup_tiles):
                        psA = opA.tile([128, SUP], f32, tag="opsA")
                        psB = opB.tile([128, SUP], f32, tag="opsB")
                        for k0, kw in _splits512(w):
                            nc.tensor.matmul(
                                psA[:, k0 : k0 + kw], lhsA,
                                w_dup[0:DH, v0 + k0 : v0 + k0 + kw],
                                start=True, stop=True,
                            )
                            nc.tensor.matmul(
                                psB[:, k0 : k0 + kw], lhsB,
                                w_dup[DH:128, v0 + k0 : v0 + k0 + kw],
                                start=True, stop=True,
                            )
                        escA = escp.tile([128, SUP], f32, tag="escA")
                        nc.scalar.activation(
                            escA[:, 0:w], psA[:, 0:w], Act.Exp,
                            accum_out=sumsA[:, sti : sti + 1],
                        )
                        escB = escp.tile([128, SUP], f32, tag="escB")
                        nc.scalar.activation(
                            escB[:, 0:w], psB[:, 0:w], Act.Exp,
                            accum_out=sumsB[:, sti : sti + 1],
                        )
                    lzs = []
                    for sums, tagp in ((sumsA, "A"), (sumsB, "B")):
                        z = statp.tile([128, 1], f32, tag="z" + tagp)
                        nc.vector.tensor_reduce(
                            z[:], sums[:, 0:ns],
                            axis=mybir.AxisListType.X, op=mybir.AluOpType.add,
                        )
                        # logZ via exponent-field estimate + 3 Newton steps
                        # (exp-only -> no ACT table switch)
                        y = statp.tile([128, 1], f32, tag="y" + tagp)
                        nc.vector.tensor_scalar(
                            out=y[:], in0=z[:, 0:1].bitcast(i32u),
                            scalar1=LN2 / (1 << 23), scalar2=-LN2 * 126.955,
                            op0=mybir.AluOpType.mult, op1=mybir.AluOpType.add,
                        )
                        for _ in range(3):
                            e = statp.tile([128, 1], f32, tag="e" + tagp)
                            nc.scalar.activation(e[:], y[:], Act.Exp, scale=-1.0)
                            tmz = statp.tile([128, 1], f32, tag="t" + tagp)
                            nc.vector.tensor_tensor(
                                out=tmz[:], in0=e[:], in1=z[:],
                                op=mybir.AluOpType.mult,
                            )
                            yn = statp.tile([128, 1], f32, tag="y" + tagp)
                            nc.vector.tensor_tensor(
                                out=yn[:], in0=y[:], in1=tmz[:],
                                op=mybir.AluOpType.add,
                            )
                            y = yn
                            nc.vector.tensor_scalar_add(y[:], y[:], -1.0)
                        lzs.append(y)
                    lzA, lzB = lzs
                    sg = None
                    for sti, (v0, w) in enumerate(sup_tiles):
                        psA = opA.tile([128, SUP], f32, tag="opsA")
                        psB = opB.tile([128, SUP], f32, tag="opsB")
                        for k0, kw in _splits512(w):
                            nc.tensor.matmul(
                                psA[:, k0 : k0 + kw], lhsA,
                                w_dup[0:DH, v0 + k0 : v0 + k0 + kw],
                                start=True, stop=True,
                            )
                            nc.tensor.matmul(
                                psB[:, k0 : k0 + kw], lhsB,
                                w_dup[DH:128, v0 + k0 : v0 + k0 + kw],
                                start=True, stop=True,
                            )
                        if sg is None:
                            stgA = stp.tile([128, 2 * SUP], f32, tag="stgA")
                            stgB = stp.tile([128, 2 * SUP], f32, tag="stgB")
                            sg = (v0, stgA, stgB)
                        g0, stgA, stgB = sg
                        o0 = v0 - g0
                        nc.vector.tensor_scalar_sub(
                            stgA[:, o0 : o0 + w], psA[:, 0:w], lzA[:, 0:1]
                        )
                        nc.vector.tensor_scalar_sub(
                            stgB[:, o0 : o0 + w], psB[:, 0:w], lzB[:, 0:1]
                        )
                        if sti == ns - 1 or o0 + w >= 2 * SUP:
                            gw = o0 + w
                            nc.sync.dma_start(
                                out_d[chA * 128 : (chA + 1) * 128, g0 : g0 + gw],
                                stgA[:, 0:gw],
                            )
                            nc.sync.dma_start(
                                out_d[chB * 128 : (chB + 1) * 128, g0 : g0 + gw],
                                stgB[:, 0:gw],
                            )
                            sg = None

    nc.compile()
    return nc


def prep_host_inputs(inputs, s=S, bl=BL, v=V, ncores=NCORES):
    """Slice/repack the full inputs into one in_map per core."""
    ib = np.asarray(inputs["input_batch"]).astype(np.int32)        # (s, B)
    emb = np.ascontiguousarray(np.asarray(inputs["embedding"], dtype=np.float32))
    W_lr = np.asarray(inputs["W_ih_lr"], dtype=np.float32)          # (E+H, H)
    b_lr = np.asarray(inputs["b_ih_lr"], dtype=np.float32)          # (1, H)
    W_rl = np.asarray(inputs["W_ih_rl"], dtype=np.float32)
    b_rl = np.asarray(inputs["b_ih_rl"], dtype=np.float32)
    W_ho = np.asarray(inputs["W_ho"], dtype=np.float32)             # (2H, v)
    b_ho = np.asarray(inputs["b_ho"], dtype=np.float32)             # (1, v)
    init = np.asarray(inputs["initial_hidden"], dtype=np.float32)   # (1, H)

    r = s * bl
    nch = r // 128
    c_init = C_INIT + bl

    w_aug = np.zeros((DH, v), np.float16)
    w_aug[0:H] = W_ho[0:H].astype(np.float16)
    w_aug[HP : HP + H] = W_ho[H : 2 * H].astype(np.float16)
    w_aug[LANE] = b_ho[0].astype(np.float16)      # lane value is exactly 1.0
    w_dup = np.ascontiguousarray(np.concatenate([w_aug, w_aug], axis=0))

    s16 = np.zeros((128, c_init), np.float16)
    s16[:, C_WLRH : C_WLRH + H] = W_lr[:EH]
    s16[:, C_WRLH : C_WRLH + H] = W_rl[:EH]
    s16[0:EL, C_WLRL : C_WLRL + H] = W_lr[EH:E]
    s16[0:EL, C_WRLL : C_WRLL + H] = W_rl[EH:E]
    s16[0:H, C_WHB : C_WHB + H] = W_lr[E : E + H]
    s16[HP : HP + H, C_WHB + HP : C_WHB + HP + H] = W_rl[E : E + H]
    s16[0:DH, C_I64B : C_I64B + DH] = np.eye(DH, dtype=np.float16)
    s16[DH, C_I64B : C_I64B + H] = b_lr[0]        # bias row (via ones in xprojT)
    s16[DH, C_I64B + HP : C_I64B + HP + H] = b_rl[0]
    s16[DH, C_I64B + LANE] = 8.0                  # tanh(8) == 1.0 in fp16
    s16[0:H, C_INIT : c_init] = init.T
    s16[HP : HP + H, C_INIT : c_init] = init.T
    s16[LANE, C_INIT : c_init] = 1.0              # lane state in init too

    s32 = np.zeros((128, 128), np.float32)
    s32[:, 0:128] = np.eye(128, dtype=np.float32)

    shared = {"emb": emb, "w_dup": w_dup, "smalls16": s16, "smalls32": s32}
    in_maps = []
    for c in range(ncores):
        ibc = ib[:, c * bl : (c + 1) * bl]                    # (s, bl)
        flat_lr = ibc.reshape(-1)                             # r = t*bl + b
        flat_rl = ibc[::-1].reshape(-1)
        idxp = np.empty((128, 2 * nch), np.int32)
        idxp[:, 0:nch] = flat_lr.reshape(nch, 128).T
        idxp[:, nch : 2 * nch] = flat_rl.reshape(nch, 128).T
        in_maps.append(dict(shared, idx=idxp))
    return in_maps


_CACHED = {}


def _get_program():
    if "nc" not in _CACHED:
        _CACHED["nc"] = build_program()
    return _CACHED["nc"]


def _unshard(res, s, bl, v, ncores):
    out = np.empty((s, ncores * bl, v), np.float32)
    nch = (s * bl) // 128
    _, pairs = _chunk_map(s, bl, nch)
    for c in range(ncores):
        out[:, c * bl : (c + 1) * bl, :] = res[c]["out"].reshape(s, bl, v)
    return out


def run_on_hw(inputs, trace=False):
    from concourse.bass_utils import run_bass_kernel_spmd

    nc = _get_program()
    in_maps = prep_host_inputs(inputs)
    res = run_bass_kernel_spmd(
        nc, in_maps, core_ids=list(range(NCORES)), trace=trace
    )
    return _unshard(res.results, S, BL, V, NCORES), res


def kernel(**inputs):
    out, _ = run_on_hw(inputs, trace=False)
    return out


# revision 8
# speedup vs baseline: 1.0098x; 1.0098x over previous
"""BiRNN language-model kernel for 8 Trainium2 NeuronCores.

Strategy: data-parallel over the batch dim (B=32 -> 4 per core), no
collectives.  Per core:
  1. indirect-DMA gather of the core's S*4 embedding rows (natural order
     for the L->R scan, time-reversed order for the R->L scan)
  2. per-128-token-chunk: PE transposes -> x-projection matmuls into
     xpL/xpR[33, S*4] (rows 0:30 = W_e^T emb per direction, row 32 =
     ones), pre-injected together with the input biases and a constant
     tanh(8)==1 lane (which later carries b_ho) into two PSUM banks.
  3. sequential scan as TWO independent chains (L->R and R->L), each
     step ONE accumulating [32,32]@[32,4] fp16 matmul + tanh that writes
     its hcat slice directly; the chains interleave on PE/ACT so the
     effective step cost is roughly halved vs a fused chain.
  4. output projection + log_softmax over V=32000 in two passes:
     pass1 logits -> PSUM, ACT exp with accum_out row sums; logZ = ln(Z)
     via an exponent-field estimate + 3 exp-Newton steps (exp only -- no
     ACT table switch, tanh/exp share one set).  pass2 recomputes logits
     and subtracts logZ during the PSUM->SBUF copy; 1MB DMA stores.

Hardware notes this shape exploits (measured here):
  - fp32 matmuls run 4 cycles/row (LOW_HIGH); fp16/bf16 run 1 cycle/row
    BUT only when the operands span 128 partitions -- K=64, N=512 fp16
    matmuls run at HALF rate.  Hence the output matmuls use K=128 with
    the top 64 weight rows zeroed, and the hidden states stored twice
    (hcatP1 and a half-swapped hcatP2) so each 128-row chunk's logits
    come from one full-partition matmul at 216ns.
  - SBUF access patterns must start at partition 0/32/64/96; direction
    blocks are padded 30->32 rows (zero weight rows kill the pads).
  - measured end-to-end rel err ~5e-5 (fp16 operands, fp32 accumulate).
"""

import sys

import numpy as np

for _p in ("/opt/trn_rl_repo", "/root/.axon_site/_ro/trn_rl_repo"):
    if _p not in sys.path:
        sys.path.insert(0, _p)

# problem constants
S, B, V, E, H = 128, 32, 32000, 150, 30
NCORES = 8
BL = B // NCORES          # batch rows per core
HP = 32                   # H padded to the 32-partition alignment
DH = 2 * HP               # 64: stacked direction state rows per chunk-half
LANE = 62                 # constant-one lane (carries b_ho): RL pad row 30
EH = 128                  # embedding dims handled by the "hi" K-split
EL = E - EH               # 22 remaining dims
VS = 512                  # fp32 matmul free-dim max (one PSUM bank)
SUP = 2048                # supertile: 4 PSUM banks, one ACT/DVE/DMA op
LN2 = float(np.log(2.0))

# packed "smalls16" column layout (fp16, [128, n]):
#  whL dup'd at rows 0:32 & 64:96; whR dup'd at rows 32:64 & 96:128
C_WLRH, C_WRLH, C_WLRL, C_WRLL = 0, 30, 60, 90
C_WH, C_ILB, C_IRB, C_INIT = 120, 152, 184, 216
C_S16 = C_INIT + BL


def _v_supertiles(v_total):
    tiles = []
    v0 = 0
    while v0 < v_total:
        w = min(SUP, v_total - v0)
        tiles.append((v0, w))
        v0 += w
    return tiles


def _splits512(w):
    out = []
    k0 = 0
    while k0 < w:
        kw = min(VS, w - k0)
        out.append((k0, kw))
        k0 += kw
    return out


def _chunk_map(s, bl, nch):
    """chunk -> (half, window) of hcatP1, ordered by scan-readiness."""
    tw = 128 // bl
    ready = lambda ch: max(tw * ch + tw - 2, s - 2 - tw * ch)
    order = sorted(range(nch), key=ready)
    cmap = {ch: (pos % 2, pos // 2) for pos, ch in enumerate(order)}
    return cmap, order


def build_program(s=S, bl=BL, v=V):
    """Build the per-core Bass program (identical on all cores)."""
    from concourse import bacc, mybir
    import concourse.tile as tile

    f32 = mybir.dt.float32
    f16 = mybir.dt.float16
    i32 = mybir.dt.int32
    Act = mybir.ActivationFunctionType

    r = s * bl                 # rows per core
    nch = r // 128             # 128-row chunks
    tw = 128 // bl             # tokens per chunk
    assert r % 256 == 0, "need an even number of 128-row chunks"
    sup_tiles = _v_supertiles(v)
    ns = len(sup_tiles)
    cmap, order = _chunk_map(s, bl, nch)
    c_init = C_INIT + bl

    nc = bacc.Bacc(None, target_bir_lowering=False)

    idx_d = nc.dram_tensor("idx", [128, 2 * nch], i32, kind="ExternalInput")
    emb_d = nc.dram_tensor("emb", [V, E], f32, kind="ExternalInput")
    w_dup_d = nc.dram_tensor("w_dup", [128, v], f16, kind="ExternalInput")
    s16_d = nc.dram_tensor("smalls16", [128, c_init], f16, kind="ExternalInput")
    s32_d = nc.dram_tensor("smalls32", [128, 128], f32, kind="ExternalInput")
    out_d = nc.dram_tensor("out", [r, v], f32, kind="ExternalOutput")

    from concourse import bass

    with tile.TileContext(nc) as tc:
        with (
            tc.tile_pool(name="persist", bufs=1) as pp,
            tc.tile_pool(name="stage", bufs=3) as stp,
            tc.tile_pool(name="esc", bufs=2) as escp,
            tc.tile_pool(name="stat", bufs=4) as statp,
        ):
            # ---- input loads (idx first: the gather chain is the long pole)
            idx = pp.tile([128, 2 * nch], i32)
            nc.sync.dma_start(idx[:], idx_d[:])
            s16 = pp.tile([128, c_init], f16)
            nc.sync.dma_start(s16[:], s16_d[:])
            s32 = pp.tile([128, 128], f32)
            nc.sync.dma_start(s32[:], s32_d[:])
            w_dup = pp.tile([128, v], f16)
            nc.sync.dma_start(w_dup[:], w_dup_d[:])

            ident = s32[:, 0:128]
            we_lr_hi = s16[:, C_WLRH : C_WLRH + H]
            we_rl_hi = s16[:, C_WRLH : C_WRLH + H]
            we_lr_lo = s16[0:EL, C_WLRL : C_WLRL + H]
            we_rl_lo = s16[0:EL, C_WRLL : C_WRLL + H]
            whL = {0: s16[0:HP, C_WH : C_WH + HP], 64: s16[64:96, C_WH : C_WH + HP]}
            whR = {32: s16[HP:DH, C_WH : C_WH + HP], 96: s16[96:128, C_WH : C_WH + HP]}
            iLb = s16[0 : HP + 1, C_ILB : C_ILB + HP]
            iRb = s16[0 : HP + 1, C_IRB : C_IRB + HP]
            init_sb = s16[0:DH, C_INIT : C_INIT + bl]

            # ---- gathers (all issued up front; chunks stream through) -----
            embg_lr = pp.tile([128, nch, E], f32)
            embg_rl = pp.tile([128, nch, E], f32)
            for j in range(nch):
                nc.gpsimd.indirect_dma_start(
                    out=embg_lr[:, j, :], out_offset=None, in_=emb_d[:],
                    in_offset=bass.IndirectOffsetOnAxis(ap=idx[:, j : j + 1], axis=0),
                )
                nc.gpsimd.indirect_dma_start(
                    out=embg_rl[:, j, :], out_offset=None, in_=emb_d[:],
                    in_offset=bass.IndirectOffsetOnAxis(
                        ap=idx[:, nch + j : nch + j + 1], axis=0
                    ),
                )

            embT_hi_lr = pp.tile([EH, r], f16)
            embT_hi_rl = pp.tile([EH, r], f16)
            embT_lo_lr = pp.tile([EL, r], f16)
            embT_lo_rl = pp.tile([EL, r], f16)

            xpL = pp.tile([HP + 1, r], f16)      # row 32 = ones (bias inject)
            nc.vector.memset(xpL[:], 0.0)
            nc.vector.memset(xpL[HP : HP + 1, :], 1.0)
            xpR = pp.tile([HP + 1, r], f16)
            nc.vector.memset(xpR[:], 0.0)
            nc.vector.memset(xpR[HP : HP + 1, :], 1.0)

            nwin = nch // 2
            hcatP1 = pp.tile([128, nwin * 128], f16)
            nc.vector.memset(hcatP1[:], 0.0)
            hcatP2 = pp.tile([128, nwin * 128], f16)
            nc.vector.memset(hcatP2[64:128, :], 0.0)

            # init states: hLR[0] -> chunk 0 col 0, hRL[s] -> chunk nch-1 col 127
            h0, w0 = cmap[0]
            nc.vector.tensor_copy(
                hcatP1[h0 * 64 : h0 * 64 + HP, w0 * 128 : w0 * 128 + bl],
                init_sb[0:HP, :],
            )
            h1, w1 = cmap[nch - 1]
            nc.vector.tensor_copy(
                hcatP1[h1 * 64 + HP : h1 * 64 + DH,
                       w1 * 128 + 128 - bl : w1 * 128 + 128],
                init_sb[HP:DH, :],
            )

            def lr_loc(i):
                """(rows, cols) of hLR[i] in hcatP1."""
                hh, ww = cmap[i // tw]
                return hh * 64, ww * 128 + (i % tw) * bl

            def rl_loc(i):
                """(rows, cols) of hRL[i+1] in hcatP1."""
                hh, ww = cmap[i // tw]
                return hh * 64 + HP, ww * 128 + (i % tw) * bl

            # ---- chunk-pipelined: transpose -> xproj -> prefill -> scan ----
            with (
                tc.tile_pool(name="pre_psum", bufs=2, space="PSUM") as prepsum,
                tc.tile_pool(name="xp_psum", bufs=2, space="PSUM") as xpp,
                tc.tile_pool(name="scanL", bufs=1, space="PSUM") as scL,
                tc.tile_pool(name="scanR", bufs=1, space="PSUM") as scR,
            ):
                pscanL = scL.tile([HP, VS], f32)
                pscanR = scR.tile([HP, VS], f32)
                for ch in range(nch):
                    cs = slice(ch * 128, (ch + 1) * 128)
                    for embg, ehi, elo in (
                        (embg_lr, embT_hi_lr, embT_lo_lr),
                        (embg_rl, embT_hi_rl, embT_lo_rl),
                    ):
                        tp = prepsum.tile([128, 128], f32, tag="tp")
                        nc.tensor.transpose(tp[:], embg[:, ch, 0:EH], ident)
                        nc.vector.tensor_copy(ehi[:, cs], tp[:])
                        tp2 = prepsum.tile([128, 128], f32, tag="tp")
                        nc.tensor.transpose(tp2[0:EL, :], embg[:, ch, EH:E], ident)
                        nc.vector.tensor_copy(elo[:, cs], tp2[0:EL, :])
                    for xp, whi, wlo, ehi, elo in (
                        (xpL, we_lr_hi, we_lr_lo, embT_hi_lr, embT_lo_lr),
                        (xpR, we_rl_hi, we_rl_lo, embT_hi_rl, embT_lo_rl),
                    ):
                        psx = xpp.tile([H, 128], f32, tag="xp")
                        nc.tensor.matmul(psx[:], whi, ehi[:, cs], start=True, stop=False)
                        nc.tensor.matmul(psx[:], wlo, elo[:, cs], start=False, stop=True)
                        nc.vector.tensor_copy(xp[0:H, cs], psx[:])
                    # prefill both chains' pre-activations (+bias, +8.0 lane)
                    pc0 = ch * 128
                    pcw = min(128, (s - 1) * bl - pc0)
                    if pcw > 0:
                        nc.tensor.matmul(
                            pscanL[:, pc0 : pc0 + pcw], iLb, xpL[:, pc0 : pc0 + pcw],
                            start=(ch == 0), stop=False, skip_group_check=True,
                        )
                        nc.tensor.matmul(
                            pscanR[:, pc0 : pc0 + pcw], iRb, xpR[:, pc0 : pc0 + pcw],
                            start=(ch == 0), stop=False, skip_group_check=True,
                        )
                    # scan steps whose xproj lives in this chunk
                    for t in range(ch * tw, min((ch + 1) * tw, s - 1)):
                        sl = slice(t * bl, (t + 1) * bl)
                        # L chain: hLR[t+1] = tanh(whL^T hLR[t] + xpL[t])
                        rr, rc = lr_loc(t)
                        nc.tensor.matmul(
                            pscanL[:, sl], whL[rr], hcatP1[rr : rr + HP, rc : rc + bl],
                            start=False, stop=(t == s - 2), skip_group_check=True,
                            tile_position=(rr, 0),
                        )
                        dr, dc = lr_loc(t + 1)
                        nc.scalar.activation(
                            hcatP1[dr : dr + HP, dc : dc + bl], pscanL[:, sl], Act.Tanh
                        )
                        # R chain: hRL[s-1-t] = tanh(whR^T hRL[s-t] + xpR_rev[t])
                        rr, rc = rl_loc(s - 1 - t)
                        nc.tensor.matmul(
                            pscanR[:, sl], whR[rr], hcatP1[rr : rr + HP, rc : rc + bl],
                            start=False, stop=(t == s - 2), skip_group_check=True,
                            tile_position=(rr, 0),
                        )
                        dr, dc = rl_loc(s - 2 - t)
                        nc.scalar.activation(
                            hcatP1[dr : dr + HP, dc : dc + bl], pscanR[:, sl], Act.Tanh
                        )

            # half-swapped copy: window w of hcatP2 rows 0:64 = hcatP1 rows 64:128
            for w_ in range(nwin):
                nc.vector.tensor_copy(
                    hcatP2[0:64, w_ * 128 : (w_ + 1) * 128],
                    hcatP1[64:128, w_ * 128 : (w_ + 1) * 128],
                )

            # ---- output projection + log_softmax ----------------------------
            with tc.tile_pool(name="out_psum", bufs=2, space="PSUM") as opsum:
                for mi, ch in enumerate(order):
                    half, win = cmap[ch]
                    src = hcatP1 if half == 0 else hcatP2
                    lhs = src[:, win * 128 : (win + 1) * 128]
                    sums = statp.tile([128, ns], f32, tag="sums")
                    for sti, (v0, w) in enumerate(sup_tiles):
                        ps = opsum.tile([128, SUP], f32, tag="ops")
                        for k0, kw in _splits512(w):
                            nc.tensor.matmul(
                                ps[:, k0 : k0 + kw], lhs,
                                w_dup[:, v0 + k0 : v0 + k0 + kw],
                                start=True, stop=True,
                            )
                        esc = escp.tile([128, SUP], f32, tag="esc")
                        nc.scalar.activation(
                            esc[:, 0:w], ps[:, 0:w], Act.Exp,
                            accum_out=sums[:, sti : sti + 1],
                        )
                    z = statp.tile([128, 1], f32, tag="z")
                    nc.vector.tensor_reduce(
                        z[:], sums[:, 0:ns],
                        axis=mybir.AxisListType.X, op=mybir.AluOpType.add,
                    )
                    # logZ via exponent-field estimate + 3 Newton steps
                    y = statp.tile([128, 1], f32, tag="y")
                    nc.vector.tensor_scalar(
                        out=y[:], in0=z[:, 0:1].bitcast(i32),
                        scalar1=LN2 / (1 << 23), scalar2=-LN2 * 126.955,
                        op0=mybir.AluOpType.mult, op1=mybir.AluOpType.add,
                    )
                    for _ in range(3):
                        e = statp.tile([128, 1], f32, tag="e")
                        nc.scalar.activation(e[:], y[:], Act.Exp, scale=-1.0)
                        tmz = statp.tile([128, 1], f32, tag="t")
                        nc.vector.tensor_tensor(
                            out=tmz[:], in0=e[:], in1=z[:], op=mybir.AluOpType.mult
                        )
                        yn = statp.tile([128, 1], f32, tag="y")
                        nc.vector.tensor_tensor(
                            out=yn[:], in0=y[:], in1=tmz[:], op=mybir.AluOpType.add
                        )
                        y = yn
                        nc.vector.tensor_scalar_add(y[:], y[:], -1.0)
                    for gi in range(0, ns, 2):
                        grp = sup_tiles[gi : gi + 2]
                        g0 = grp[0][0]
                        gw = sum(g[1] for g in grp)
                        stg = stp.tile([128, 2 * SUP], f32, tag="stg")
                        for v0, w in grp:
                            ps = opsum.tile([128, SUP], f32, tag="ops")
                            for k0, kw in _splits512(w):
                                nc.tensor.matmul(
                                    ps[:, k0 : k0 + kw], lhs,
                                    w_dup[:, v0 + k0 : v0 + k0 + kw],
                                    start=True, stop=True,
                                )
                            nc.vector.tensor_scalar_sub(
                                stg[:, v0 - g0 : v0 - g0 + w], ps[:, 0:w], y[:, 0:1]
                            )
                        nc.sync.dma_start(
                            out_d[ch * 128 : (ch + 1) * 128, g0 : g0 + gw],
                            stg[:, 0:gw],
                        )

    nc.compile()
    return nc


def prep_host_inputs(inputs, s=S, bl=BL, v=V, ncores=NCORES):
    """Slice/repack the full inputs into one in_map per core."""
    ib = np.asarray(inputs["input_batch"]).astype(np.int32)        # (s, B)
    emb = np.ascontiguousarray(np.asarray(inputs["embedding"], dtype=np.float32))
    W_lr = np.asarray(inputs["W_ih_lr"], dtype=np.float32)          # (E+H, H)
    b_lr = np.asarray(inputs["b_ih_lr"], dtype=np.float32)          # (1, H)
    W_rl = np.asarray(inputs["W_ih_rl"], dtype=np.float32)
    b_rl = np.asarray(inputs["b_ih_rl"], dtype=np.float32)
    W_ho = np.asarray(inputs["W_ho"], dtype=np.float32)             # (2H, v)
    b_ho = np.asarray(inputs["b_ho"], dtype=np.float32)             # (1, v)
    init = np.asarray(inputs["initial_hidden"], dtype=np.float32)   # (1, H)

    r = s * bl
    nch = r // 128
    c_init = C_INIT + bl

    w_dup = np.zeros((128, v), np.float16)
    w_dup[0:H] = W_ho[0:H].astype(np.float16)
    w_dup[HP : HP + H] = W_ho[H : 2 * H].astype(np.float16)
    w_dup[LANE] = b_ho[0].astype(np.float16)      # lane value is exactly 1.0

    s16 = np.zeros((128, c_init), np.float16)
    s16[:, C_WLRH : C_WLRH + H] = W_lr[:EH]
    s16[:, C_WRLH : C_WRLH + H] = W_rl[:EH]
    s16[0:EL, C_WLRL : C_WLRL + H] = W_lr[EH:E]
    s16[0:EL, C_WRLL : C_WRLL + H] = W_rl[EH:E]
    # scan weights, dup'd for both partition bases
    s16[0:H, C_WH : C_WH + H] = W_lr[E : E + H]
    s16[64 : 64 + H, C_WH : C_WH + H] = W_lr[E : E + H]
    s16[HP : HP + H, C_WH : C_WH + H] = W_rl[E : E + H]
    s16[96 : 96 + H, C_WH : C_WH + H] = W_rl[E : E + H]
    # identity-plus-bias prefill weights
    s16[0:HP, C_ILB : C_ILB + HP] = np.eye(HP, dtype=np.float16)
    s16[HP, C_ILB : C_ILB + H] = b_lr[0]
    s16[0:HP, C_IRB : C_IRB + HP] = np.eye(HP, dtype=np.float16)
    s16[HP, C_IRB : C_IRB + H] = b_rl[0]
    s16[HP, C_IRB + H] = 8.0                      # tanh(8) == 1.0 in fp16 (lane)
    s16[0:H, C_INIT : c_init] = init.T
    s16[HP : HP + H, C_INIT : c_init] = init.T
    s16[LANE, C_INIT : c_init] = 1.0              # lane state in init too

    s32 = np.zeros((128, 128), np.float32)
    s32[:, 0:128] = np.eye(128, dtype=np.float32)

    shared = {"emb": emb, "w_dup": w_dup, "smalls16": s16, "smalls32": s32}
    in_maps = []
    for c in range(ncores):
        ibc = ib[:, c * bl : (c + 1) * bl]                    # (s, bl)
        flat_lr = ibc.reshape(-1)                             # r = t*bl + b
        flat_rl = ibc[::-1].reshape(-1)
        idxp = np.empty((128, 2 * nch), np.int32)
        idxp[:, 0:nch] = flat_lr.reshape(nch, 128).T
        idxp[:, nch : 2 * nch] = flat_rl.reshape(nch, 128).T
        in_maps.append(dict(shared, idx=idxp))
    return in_maps


_CACHED = {}


def _get_program():
    if "nc" not in _CACHED:
        _CACHED["nc"] = build_program()
    return _CACHED["nc"]


def run_on_hw(inputs, trace=False):
    from concourse.bass_utils import run_bass_kernel_spmd

    nc = _get_program()
    in_maps = prep_host_inputs(inputs)
    res = run_bass_kernel_spmd(
        nc, in_maps, core_ids=list(range(NCORES)), trace=trace
    )
    out = np.empty((S, B, V), np.float32)
    for c in range(NCORES):
        out[:, c * BL : (c + 1) * BL, :] = res.results[c]["out"].reshape(S, BL, V)
    return out, res


def kernel(**inputs):
    out, _ = run_on_hw(inputs, trace=False)
    return out


# revision 9
# speedup vs baseline: 1.1600x; 1.1487x over previous
"""BiRNN language-model kernel for 8 Trainium2 NeuronCores.

Strategy: data-parallel over the batch dim (B=32 -> 4 per core), no
collectives.  Per core:
  1. indirect-DMA gather of the core's S*4 embedding rows (natural order
     for the L->R scan, time-reversed order for the R->L scan)
  2. per-128-token-chunk: PE transposes -> x-projection matmuls into
     xpL/xpR[33, S*4] (rows 0:30 = W_e^T emb per direction, row 32 =
     ones), pre-injected together with the input biases and a constant
     tanh(8)==1 lane (which later carries b_ho) into two PSUM banks.
  3. sequential scan as TWO independent chains (L->R and R->L), each
     step ONE accumulating [32,32]@[32,4] fp16 matmul + tanh that writes
     its hcat slice directly; the chains interleave on PE/ACT so the
     effective step cost is roughly halved vs a fused chain.
  4. output projection + log_softmax over V=32000 in two passes:
     pass1 logits -> PSUM, ACT exp with accum_out row sums; logZ = ln(Z)
     via an exponent-field estimate + 3 exp-Newton steps (exp only -- no
     ACT table switch, tanh/exp share one set).  pass2 recomputes logits
     and subtracts logZ during the PSUM->SBUF copy; 1MB DMA stores.

Hardware notes this shape exploits (measured here):
  - fp32 matmuls run 4 cycles/row (LOW_HIGH); fp16/bf16 run 1 cycle/row
    BUT only when the operands span 128 partitions -- K=64, N=512 fp16
    matmuls run at HALF rate.  Hence the output matmuls use K=128 with
    the top 64 weight rows zeroed, and the hidden states stored twice
    (hcatP1 and a half-swapped hcatP2) so each 128-row chunk's logits
    come from one full-partition matmul at 216ns.
  - SBUF access patterns must start at partition 0/32/64/96; direction
    blocks are padded 30->32 rows (zero weight rows kill the pads).
  - measured end-to-end rel err ~5e-5 (fp16 operands, fp32 accumulate).
"""

import sys

import numpy as np

for _p in ("/opt/trn_rl_repo", "/root/.axon_site/_ro/trn_rl_repo"):
    if _p not in sys.path:
        sys.path.insert(0, _p)

# problem constants
S, B, V, E, H = 128, 32, 32000, 150, 30
NCORES = 8
BL = B // NCORES          # batch rows per core
HP = 32                   # H padded to the 32-partition alignment
DH = 2 * HP               # 64: stacked direction state rows per chunk-half
LANE = 62                 # constant-one lane (carries b_ho): RL pad row 30
EH = 128                  # embedding dims handled by the "hi" K-split
EL = E - EH               # 22 remaining dims
VS = 512                  # fp32 matmul free-dim max (one PSUM bank)
SUP = 2048                # supertile: 4 PSUM banks, one ACT/DVE/DMA op
LN2 = float(np.log(2.0))

# packed "smalls16" column layout (fp16, [128, n]):
#  whL dup'd at rows 0:32 & 64:96; whR dup'd at rows 32:64 & 96:128
C_WLRH, C_WRLH, C_WLRL, C_WRLL = 0, 30, 60, 90
C_WH, C_ILB, C_IRB, C_INIT = 120, 152, 184, 216
C_S16 = C_INIT + BL


def _v_supertiles(v_total):
    tiles = []
    v0 = 0
    while v0 < v_total:
        w = min(SUP, v_total - v0)
        tiles.append((v0, w))
        v0 += w
    return tiles


def _splits512(w):
    out = []
    k0 = 0
    while k0 < w:
        kw = min(VS, w - k0)
        out.append((k0, kw))
        k0 += kw
    return out


def _chunk_map(s, bl, nch):
    """chunk -> (half, window) of hcatP1, ordered by scan-readiness."""
    tw = 128 // bl
    ready = lambda ch: max(tw * ch + tw - 2, s - 2 - tw * ch)
    order = sorted(range(nch), key=ready)
    cmap = {ch: (pos % 2, pos // 2) for pos, ch in enumerate(order)}
    return cmap, order


def build_program(s=S, bl=BL, v=V):
    """Build the per-core Bass program (identical on all cores)."""
    from concourse import bacc, mybir
    import concourse.tile as tile

    f32 = mybir.dt.float32
    f16 = mybir.dt.float16
    i32 = mybir.dt.int32
    Act = mybir.ActivationFunctionType

    r = s * bl                 # rows per core
    nch = r // 128             # 128-row chunks
    tw = 128 // bl             # tokens per chunk
    assert r % 256 == 0, "need an even number of 128-row chunks"
    sup_tiles = _v_supertiles(v)
    ns = len(sup_tiles)
    cmap, order = _chunk_map(s, bl, nch)
    c_init = C_INIT + bl

    nc = bacc.Bacc(None, target_bir_lowering=False)

    idx_d = nc.dram_tensor("idx", [128, 2 * nch], i32, kind="ExternalInput")
    emb_d = nc.dram_tensor("emb", [V, E], f32, kind="ExternalInput")
    w_dup_d = nc.dram_tensor("w_dup", [128, v], f16, kind="ExternalInput")
    s16_d = nc.dram_tensor("smalls16", [128, c_init], f16, kind="ExternalInput")
    s32_d = nc.dram_tensor("smalls32", [128, 128], f32, kind="ExternalInput")
    out_d = nc.dram_tensor("out", [r, v], f32, kind="ExternalOutput")

    from concourse import bass

    with tile.TileContext(nc) as tc:
        with (
            tc.tile_pool(name="persist", bufs=1) as pp,
            tc.tile_pool(name="stage", bufs=3) as stp,
            tc.tile_pool(name="esc", bufs=2) as escp,
            tc.tile_pool(name="stat", bufs=4) as statp,
        ):
            # ---- input loads (idx first: the gather chain is the long pole)
            idx = pp.tile([128, 2 * nch], i32)
            nc.sync.dma_start(idx[:], idx_d[:])
            s16 = pp.tile([128, c_init], f16)
            nc.sync.dma_start(s16[:], s16_d[:])
            s32 = pp.tile([128, 128], f32)
            nc.sync.dma_start(s32[:], s32_d[:])
            w_dup = pp.tile([128, v], f16)
            nc.sync.dma_start(w_dup[:], w_dup_d[:])

            ident = s32[:, 0:128]
            we_lr_hi = s16[:, C_WLRH : C_WLRH + H]
            we_rl_hi = s16[:, C_WRLH : C_WRLH + H]
            we_lr_lo = s16[0:EL, C_WLRL : C_WLRL + H]
            we_rl_lo = s16[0:EL, C_WRLL : C_WRLL + H]
            whL = {0: s16[0:HP, C_WH : C_WH + HP], 64: s16[64:96, C_WH : C_WH + HP]}
            whR = {32: s16[HP:DH, C_WH : C_WH + HP], 96: s16[96:128, C_WH : C_WH + HP]}
            iLb = s16[0 : HP + 1, C_ILB : C_ILB + HP]
            iRb = s16[0 : HP + 1, C_IRB : C_IRB + HP]
            init_sb = s16[0:DH, C_INIT : C_INIT + bl]

            # ---- gathers (all issued up front; chunks stream through) -----
            embg_lr = pp.tile([128, nch, E], f32)
            embg_rl = pp.tile([128, nch, E], f32)
            for j in range(nch):
                nc.gpsimd.indirect_dma_start(
                    out=embg_lr[:, j, :], out_offset=None, in_=emb_d[:],
                    in_offset=bass.IndirectOffsetOnAxis(ap=idx[:, j : j + 1], axis=0),
                )
                nc.gpsimd.indirect_dma_start(
                    out=embg_rl[:, j, :], out_offset=None, in_=emb_d[:],
                    in_offset=bass.IndirectOffsetOnAxis(
                        ap=idx[:, nch + j : nch + j + 1], axis=0
                    ),
                )

            embT_hi_lr = pp.tile([EH, r], f16)
            embT_hi_rl = pp.tile([EH, r], f16)
            embT_lo_lr = pp.tile([EL, r], f16)
            embT_lo_rl = pp.tile([EL, r], f16)

            xpL = pp.tile([HP + 1, r], f16)      # row 32 = ones (bias inject)
            nc.vector.memset(xpL[:], 0.0)
            nc.vector.memset(xpL[HP : HP + 1, :], 1.0)
            xpR = pp.tile([HP + 1, r], f16)
            nc.vector.memset(xpR[:], 0.0)
            nc.vector.memset(xpR[HP : HP + 1, :], 1.0)

            nwin = nch // 2
            hcatP1 = pp.tile([128, nwin * 128], f16)
            nc.vector.memset(hcatP1[:], 0.0)
            hcatP2 = pp.tile([128, nwin * 128], f16)
            nc.vector.memset(hcatP2[64:128, :], 0.0)

            # init states: hLR[0] -> chunk 0 col 0, hRL[s] -> chunk nch-1 col 127
            h0, w0 = cmap[0]
            nc.vector.tensor_copy(
                hcatP1[h0 * 64 : h0 * 64 + HP, w0 * 128 : w0 * 128 + bl],
                init_sb[0:HP, :],
            )
            h1, w1 = cmap[nch - 1]
            nc.vector.tensor_copy(
                hcatP1[h1 * 64 + HP : h1 * 64 + DH,
                       w1 * 128 + 128 - bl : w1 * 128 + 128],
                init_sb[HP:DH, :],
            )

            def lr_loc(i):
                """(rows, cols) of hLR[i] in hcatP1."""
                hh, ww = cmap[i // tw]
                return hh * 64, ww * 128 + (i % tw) * bl

            def rl_loc(i):
                """(rows, cols) of hRL[i+1] in hcatP1."""
                hh, ww = cmap[i // tw]
                return hh * 64 + HP, ww * 128 + (i % tw) * bl

            # ---- chunk-pipelined: transpose -> xproj -> prefill -> scan ----
            with (
                tc.tile_pool(name="pre_psum", bufs=2, space="PSUM") as prepsum,
                tc.tile_pool(name="xp_psum", bufs=2, space="PSUM") as xpp,
                tc.tile_pool(name="scanL", bufs=1, space="PSUM") as scL,
                tc.tile_pool(name="scanR", bufs=1, space="PSUM") as scR,
            ):
                pscanL = scL.tile([HP, VS], f32)
                pscanR = scR.tile([HP, VS], f32)
                for ch in range(nch):
                    cs = slice(ch * 128, (ch + 1) * 128)
                    for embg, ehi, elo in (
                        (embg_lr, embT_hi_lr, embT_lo_lr),
                        (embg_rl, embT_hi_rl, embT_lo_rl),
                    ):
                        tp = prepsum.tile([128, 128], f32, tag="tp")
                        nc.tensor.transpose(tp[:], embg[:, ch, 0:EH], ident)
                        nc.vector.tensor_copy(ehi[:, cs], tp[:])
                        tp2 = prepsum.tile([128, 128], f32, tag="tp")
                        nc.tensor.transpose(tp2[0:EL, :], embg[:, ch, EH:E], ident)
                        nc.vector.tensor_copy(elo[:, cs], tp2[0:EL, :])
                    for xp, whi, wlo, ehi, elo in (
                        (xpL, we_lr_hi, we_lr_lo, embT_hi_lr, embT_lo_lr),
                        (xpR, we_rl_hi, we_rl_lo, embT_hi_rl, embT_lo_rl),
                    ):
                        psx = xpp.tile([H, 128], f32, tag="xp")
                        nc.tensor.matmul(psx[:], whi, ehi[:, cs], start=True, stop=False)
                        nc.tensor.matmul(psx[:], wlo, elo[:, cs], start=False, stop=True)
                        nc.vector.tensor_copy(xp[0:H, cs], psx[:])
                    # prefill both chains' pre-activations (+bias, +8.0 lane)
                    pc0 = ch * 128
                    pcw = min(128, (s - 1) * bl - pc0)
                    if pcw > 0:
                        nc.tensor.matmul(
                            pscanL[:, pc0 : pc0 + pcw], iLb, xpL[:, pc0 : pc0 + pcw],
                            start=(ch == 0), stop=False, skip_group_check=True,
                        )
                        nc.tensor.matmul(
                            pscanR[:, pc0 : pc0 + pcw], iRb, xpR[:, pc0 : pc0 + pcw],
                            start=(ch == 0), stop=False, skip_group_check=True,
                        )
                    # scan steps whose xproj lives in this chunk
                    for t in range(ch * tw, min((ch + 1) * tw, s - 1)):
                        sl = slice(t * bl, (t + 1) * bl)
                        # L chain: hLR[t+1] = tanh(whL^T hLR[t] + xpL[t])
                        rr, rc = lr_loc(t)
                        nc.tensor.matmul(
                            pscanL[:, sl], whL[rr], hcatP1[rr : rr + HP, rc : rc + bl],
                            start=False, stop=(t == s - 2), skip_group_check=True,
                            tile_position=(rr, 0),
                        )
                        dr, dc = lr_loc(t + 1)
                        nc.scalar.activation(
                            hcatP1[dr : dr + HP, dc : dc + bl], pscanL[:, sl], Act.Tanh
                        )
                        # R chain: hRL[s-1-t] = tanh(whR^T hRL[s-t] + xpR_rev[t])
                        rr, rc = rl_loc(s - 1 - t)
                        nc.tensor.matmul(
                            pscanR[:, sl], whR[rr], hcatP1[rr : rr + HP, rc : rc + bl],
                            start=False, stop=(t == s - 2), skip_group_check=True,
                            tile_position=(rr, 0),
                        )
                        dr, dc = rl_loc(s - 2 - t)
                        nc.scalar.activation(
                            hcatP1[dr : dr + HP, dc : dc + bl], pscanR[:, sl], Act.Tanh
                        )

            # half-swapped copy: window w of hcatP2 rows 0:64 = hcatP1 rows 64:128
            for w_ in range(nwin):
                nc.vector.tensor_copy(
                    hcatP2[0:64, w_ * 128 : (w_ + 1) * 128],
                    hcatP1[64:128, w_ * 128 : (w_ + 1) * 128],
                )

            # ---- output projection + log_softmax ----------------------------
            # software-pipelined: pass2 of chunk m interleaves with pass1 of
            # chunk m+1 at supertile granularity so ACT (exp), DVE (sub) and
            # DMA (store) all stream concurrently.
            def lhs_of(ch):
                half, win = cmap[ch]
                t_ = hcatP1 if half == 0 else hcatP2
                return t_[:, win * 128 : (win + 1) * 128]

            def emit_p1_tile(lhs, sti, sums):
                v0, w = sup_tiles[sti]
                ps = opsum.tile([128, SUP], f32, tag="ops")
                for k0, kw in _splits512(w):
                    nc.tensor.matmul(
                        ps[:, k0 : k0 + kw], lhs,
                        w_dup[:, v0 + k0 : v0 + k0 + kw],
                        start=True, stop=True,
                    )
                esc = escp.tile([128, SUP], f32, tag="esc")
                nc.scalar.activation(
                    esc[:, 0:w], ps[:, 0:w], Act.Exp,
                    accum_out=sums[:, sti : sti + 1],
                )

            def emit_newton(sums):
                z = statp.tile([128, 1], f32, tag="z")
                nc.vector.tensor_reduce(
                    z[:], sums[:, 0:ns],
                    axis=mybir.AxisListType.X, op=mybir.AluOpType.add,
                )
                y = statp.tile([128, 1], f32, tag="y")
                nc.vector.tensor_scalar(
                    out=y[:], in0=z[:, 0:1].bitcast(i32),
                    scalar1=LN2 / (1 << 23), scalar2=-LN2 * 126.955,
                    op0=mybir.AluOpType.mult, op1=mybir.AluOpType.add,
                )
                for _ in range(3):
                    e = statp.tile([128, 1], f32, tag="e")
                    nc.scalar.activation(e[:], y[:], Act.Exp, scale=-1.0)
                    tmz = statp.tile([128, 1], f32, tag="t")
                    nc.vector.tensor_tensor(
                        out=tmz[:], in0=e[:], in1=z[:], op=mybir.AluOpType.mult
                    )
                    yn = statp.tile([128, 1], f32, tag="y")
                    nc.vector.tensor_tensor(
                        out=yn[:], in0=y[:], in1=tmz[:], op=mybir.AluOpType.add
                    )
                    y = yn
                    nc.vector.tensor_scalar_add(y[:], y[:], -1.0)
                return y

            with tc.tile_pool(name="out_psum", bufs=2, space="PSUM") as opsum:
                state = {}  # per-chunk: sums, y, staging group
                sums0 = statp.tile([128, ns], f32, tag="sums0")
                for sti in range(ns):
                    emit_p1_tile(lhs_of(order[0]), sti, sums0)
                y_cur = emit_newton(sums0)
                for i, ch in enumerate(order):
                    lhs = lhs_of(ch)
                    nxt = order[i + 1] if i + 1 < nch else None
                    if nxt is not None:
                        sums_n = statp.tile([128, ns], f32, tag="sums1")
                    sg = None
                    for sti, (v0, w) in enumerate(sup_tiles):
                        # pass2 supertile of current chunk
                        ps = opsum.tile([128, SUP], f32, tag="ops")
                        for k0, kw in _splits512(w):
                            nc.tensor.matmul(
                                ps[:, k0 : k0 + kw], lhs,
                                w_dup[:, v0 + k0 : v0 + k0 + kw],
                                start=True, stop=True,
                            )
                        if sg is None:
                            stg = stp.tile([128, 2 * SUP], f32, tag="stg")
                            sg = (v0, stg)
                        g0, stg = sg
                        nc.vector.tensor_scalar_sub(
                            stg[:, v0 - g0 : v0 - g0 + w], ps[:, 0:w], y_cur[:, 0:1]
                        )
                        if sti == ns - 1 or v0 - g0 + w >= 2 * SUP:
                            gw = v0 - g0 + w
                            nc.sync.dma_start(
                                out_d[ch * 128 : (ch + 1) * 128, g0 : g0 + gw],
                                stg[:, 0:gw],
                            )
                            sg = None
                        # pass1 supertile of next chunk rides along
                        if nxt is not None:
                            emit_p1_tile(lhs_of(nxt), sti, sums_n)
                    if nxt is not None:
                        y_cur = emit_newton(sums_n)

    nc.compile()
    return nc


def prep_host_inputs(inputs, s=S, bl=BL, v=V, ncores=NCORES):
    """Slice/repack the full inputs into one in_map per core."""
    ib = np.asarray(inputs["input_batch"]).astype(np.int32)        # (s, B)
    emb = np.ascontiguousarray(np.asarray(inputs["embedding"], dtype=np.float32))
    W_lr = np.asarray(inputs["W_ih_lr"], dtype=np.float32)          # (E+H, H)
    b_lr = np.asarray(inputs["b_ih_lr"], dtype=np.float32)          # (1, H)
    W_rl = np.asarray(inputs["W_ih_rl"], dtype=np.float32)
    b_rl = np.asarray(inputs["b_ih_rl"], dtype=np.float32)
    W_ho = np.asarray(inputs["W_ho"], dtype=np.float32)             # (2H, v)
    b_ho = np.asarray(inputs["b_ho"], dtype=np.float32)             # (1, v)
    init = np.asarray(inputs["initial_hidden"], dtype=np.float32)   # (1, H)

    r = s * bl
    nch = r // 128
    c_init = C_INIT + bl

    w_dup = np.zeros((128, v), np.float16)
    w_dup[0:H] = W_ho[0:H].astype(np.float16)
    w_dup[HP : HP + H] = W_ho[H : 2 * H].astype(np.float16)
    w_dup[LANE] = b_ho[0].astype(np.float16)      # lane value is exactly 1.0

    s16 = np.zeros((128, c_init), np.float16)
    s16[:, C_WLRH : C_WLRH + H] = W_lr[:EH]
    s16[:, C_WRLH : C_WRLH + H] = W_rl[:EH]
    s16[0:EL, C_WLRL : C_WLRL + H] = W_lr[EH:E]
    s16[0:EL, C_WRLL : C_WRLL + H] = W_rl[EH:E]
    # scan weights, dup'd for both partition bases
    s16[0:H, C_WH : C_WH + H] = W_lr[E : E + H]
    s16[64 : 64 + H, C_WH : C_WH + H] = W_lr[E : E + H]
    s16[HP : HP + H, C_WH : C_WH + H] = W_rl[E : E + H]
    s16[96 : 96 + H, C_WH : C_WH + H] = W_rl[E : E + H]
    # identity-plus-bias prefill weights
    s16[0:HP, C_ILB : C_ILB + HP] = np.eye(HP, dtype=np.float16)
    s16[HP, C_ILB : C_ILB + H] = b_lr[0]
    s16[0:HP, C_IRB : C_IRB + HP] = np.eye(HP, dtype=np.float16)
    s16[HP, C_IRB : C_IRB + H] = b_rl[0]
    s16[HP, C_IRB + H] = 8.0                      # tanh(8) == 1.0 in fp16 (lane)
    s16[0:H, C_INIT : c_init] = init.T
    s16[HP : HP + H, C_INIT : c_init] = init.T
    s16[LANE, C_INIT : c_init] = 1.0              # lane state in init too

    s32 = np.zeros((128, 128), np.float32)
    s32[:, 0:128] = np.eye(128, dtype=np.float32)

    shared = {"emb": emb, "w_dup": w_dup, "smalls16": s16, "smalls32": s32}
    in_maps = []
    for c in range(ncores):
        ibc = ib[:, c * bl : (c + 1) * bl]                    # (s, bl)
        flat_lr = ibc.reshape(-1)                             # r = t*bl + b
        flat_rl = ibc[::-1].reshape(-1)
        idxp = np.empty((128, 2 * nch), np.int32)
        idxp[:, 0:nch] = flat_lr.reshape(nch, 128).T
        idxp[:, nch : 2 * nch] = flat_rl.reshape(nch, 128).T
        in_maps.append(dict(shared, idx=idxp))
    return in_maps


_CACHED = {}


def _get_program():
    if "nc" not in _CACHED:
        _CACHED["nc"] = build_program()
    return _CACHED["nc"]


def run_on_hw(inputs, trace=False):
    from concourse.bass_utils import run_bass_kernel_spmd

    nc = _get_program()
    in_maps = prep_host_inputs(inputs)
    res = run_bass_kernel_spmd(
        nc, in_maps, core_ids=list(range(NCORES)), trace=trace
    )
    out = np.empty((S, B, V), np.float32)
    for c in range(NCORES):
        out[:, c * BL : (c + 1) * BL, :] = res.results[c]["out"].reshape(S, BL, V)
    return out, res


def kernel(**inputs):
    out, _ = run_on_hw(inputs, trace=False)
    return out


# revision 11
# speedup vs baseline: 1.2228x; 1.0542x over previous
"""BiRNN language-model kernel for 8 Trainium2 NeuronCores.

Strategy: data-parallel over the batch dim (B=32 -> 4 per core), no
collectives.  Per core:
  1. indirect-DMA gather of the core's S*4 embedding rows (natural order
     for the L->R scan, time-reversed order for the R->L scan)
  2. per-128-token-chunk: PE transposes -> x-projection matmuls into
     xpL/xpR[33, S*4] (rows 0:30 = W_e^T emb per direction, row 32 =
     ones), pre-injected together with the input biases and a constant
     tanh(8)==1 lane (which later carries b_ho) into two PSUM banks.
  3. sequential scan as TWO independent chains (L->R and R->L), each
     step ONE accumulating [32,32]@[32,4] fp16 matmul + tanh that writes
     its hcat slice directly; the chains interleave on PE/ACT so the
     effective step cost is roughly halved vs a fused chain.
  4. output projection + log_softmax over V=32000 in two passes:
     pass1 logits -> PSUM, ACT exp with accum_out row sums; logZ = ln(Z)
     via an exponent-field estimate + 3 exp-Newton steps (exp only -- no
     ACT table switch, tanh/exp share one set).  pass2 recomputes logits
     and subtracts logZ during the PSUM->SBUF copy; 1MB DMA stores.

Hardware notes this shape exploits (measured here):
  - fp32 matmuls run 4 cycles/row (LOW_HIGH); fp16/bf16 run 1 cycle/row
    BUT only when the operands span 128 partitions -- K=64, N=512 fp16
    matmuls run at HALF rate.  Hence the output matmuls use K=128 with
    the top 64 weight rows zeroed, and the hidden states stored twice
    (hcatP1 and a half-swapped hcatP2) so each 128-row chunk's logits
    come from one full-partition matmul at 216ns.
  - SBUF access patterns must start at partition 0/32/64/96; direction
    blocks are padded 30->32 rows (zero weight rows kill the pads).
  - measured end-to-end rel err ~5e-5 (fp16 operands, fp32 accumulate).
"""

import sys

import numpy as np

for _p in ("/opt/trn_rl_repo", "/root/.axon_site/_ro/trn_rl_repo"):
    if _p not in sys.path:
        sys.path.insert(0, _p)

# problem constants
S, B, V, E, H = 128, 32, 32000, 150, 30
NCORES = 8
BL = B // NCORES          # batch rows per core
HP = 32                   # H padded to the 32-partition alignment
DH = 2 * HP               # 64: stacked direction state rows per chunk-half
LANE = 62                 # constant-one lane (carries b_ho): RL pad row 30
EH = 128                  # embedding dims handled by the "hi" K-split
EL = E - EH               # 22 remaining dims
VS = 512                  # fp32 matmul free-dim max (one PSUM bank)
SUP = 1024                # supertile: 2 PSUM banks per pool
LN2 = float(np.log(2.0))

# packed "smalls16" column layout (fp16, [128, n]):
#  whL dup'd at rows 0:32 & 64:96; whR dup'd at rows 32:64 & 96:128
C_WLRH, C_WRLH, C_WLRL, C_WRLL = 0, 30, 60, 90
C_WH, C_ILB, C_IRB, C_INIT = 120, 152, 184, 216
C_S16 = C_INIT + BL


def _v_supertiles(v_total):
    tiles = []
    v0 = 0
    while v0 < v_total:
        w = min(SUP, v_total - v0)
        tiles.append((v0, w))
        v0 += w
    return tiles


def _splits512(w):
    out = []
    k0 = 0
    while k0 < w:
        kw = min(VS, w - k0)
        out.append((k0, kw))
        k0 += kw
    return out


def _chunk_map(s, bl, nch):
    """chunk -> (half, window) of hcatP1, ordered by scan-readiness."""
    tw = 128 // bl
    ready = lambda ch: max(tw * ch + tw - 2, s - 2 - tw * ch)
    order = sorted(range(nch), key=ready)
    cmap = {ch: (pos % 2, pos // 2) for pos, ch in enumerate(order)}
    return cmap, order


def build_program(s=S, bl=BL, v=V):
    """Build the per-core Bass program (identical on all cores)."""
    from concourse import bacc, mybir
    import concourse.tile as tile

    f32 = mybir.dt.float32
    f16 = mybir.dt.float16
    i32 = mybir.dt.int32
    Act = mybir.ActivationFunctionType

    r = s * bl                 # rows per core
    nch = r // 128             # 128-row chunks
    tw = 128 // bl             # tokens per chunk
    assert r % 256 == 0, "need an even number of 128-row chunks"
    sup_tiles = _v_supertiles(v)
    ns = len(sup_tiles)
    cmap, order = _chunk_map(s, bl, nch)
    c_init = C_INIT + bl

    nc = bacc.Bacc(None, target_bir_lowering=False)

    idx_d = nc.dram_tensor("idx", [128, 2 * nch], i32, kind="ExternalInput")
    emb_d = nc.dram_tensor("emb", [V, E], f32, kind="ExternalInput")
    w_dup_d = nc.dram_tensor("w_dup", [128, v], f16, kind="ExternalInput")
    s16_d = nc.dram_tensor("smalls16", [128, c_init], f16, kind="ExternalInput")
    s32_d = nc.dram_tensor("smalls32", [128, 128], f16, kind="ExternalInput")
    out_d = nc.dram_tensor("out", [r, v], f32, kind="ExternalOutput")

    from concourse import bass

    with tile.TileContext(nc) as tc:
        with (
            tc.tile_pool(name="persist", bufs=1) as pp,
            tc.tile_pool(name="stage", bufs=3) as stp,
            tc.tile_pool(name="esc", bufs=2) as escp,
            tc.tile_pool(name="stat", bufs=4) as statp,
        ):
            # ---- input loads (idx first: the gather chain is the long pole)
            idx = pp.tile([128, 2 * nch], i32)
            nc.sync.dma_start(idx[:], idx_d[:])
            s16 = pp.tile([128, c_init], f16)
            nc.sync.dma_start(s16[:], s16_d[:])
            s32 = pp.tile([128, 128], f16)
            nc.sync.dma_start(s32[:], s32_d[:])
            w_dup = pp.tile([128, v], f16)
            nc.sync.dma_start(w_dup[:], w_dup_d[:])

            ident = s32[:, 0:128]
            we_lr_hi = s16[:, C_WLRH : C_WLRH + H]
            we_rl_hi = s16[:, C_WRLH : C_WRLH + H]
            we_lr_lo = s16[0:EL, C_WLRL : C_WLRL + H]
            we_rl_lo = s16[0:EL, C_WRLL : C_WRLL + H]
            whL = {0: s16[0:HP, C_WH : C_WH + HP], 64: s16[64:96, C_WH : C_WH + HP]}
            whR = {32: s16[HP:DH, C_WH : C_WH + HP], 96: s16[96:128, C_WH : C_WH + HP]}
            iLb = s16[0 : HP + 1, C_ILB : C_ILB + HP]
            iRb = s16[0 : HP + 1, C_IRB : C_IRB + HP]
            init_sb = s16[0:DH, C_INIT : C_INIT + bl]

            # ---- gathers (all issued up front; chunks stream through) -----
            embg_lr = pp.tile([128, nch, E], f16)
            embg_rl = pp.tile([128, nch, E], f16)
            for j in range(nch):
                nc.gpsimd.indirect_dma_start(
                    out=embg_lr[:, j, :], out_offset=None, in_=emb_d[:],
                    in_offset=bass.IndirectOffsetOnAxis(ap=idx[:, j : j + 1], axis=0),
                )
                nc.gpsimd.indirect_dma_start(
                    out=embg_rl[:, j, :], out_offset=None, in_=emb_d[:],
                    in_offset=bass.IndirectOffsetOnAxis(
                        ap=idx[:, nch + j : nch + j + 1], axis=0
                    ),
                )

            embT_hi_lr = pp.tile([EH, r], f16)
            embT_hi_rl = pp.tile([EH, r], f16)
            embT_lo_lr = pp.tile([EL, r], f16)
            embT_lo_rl = pp.tile([EL, r], f16)

            xpL = pp.tile([HP + 1, r], f16)      # row 32 = ones (bias inject)
            nc.vector.memset(xpL[:], 0.0)
            nc.vector.memset(xpL[HP : HP + 1, :], 1.0)
            xpR = pp.tile([HP + 1, r], f16)
            nc.vector.memset(xpR[:], 0.0)
            nc.vector.memset(xpR[HP : HP + 1, :], 1.0)

            nwin = nch // 2
            hcatP1 = pp.tile([128, nwin * 128], f16)
            nc.vector.memset(hcatP1[:], 0.0)
            hcatP2 = pp.tile([128, nwin * 128], f16)
            nc.vector.memset(hcatP2[64:128, :], 0.0)

            # init states: hLR[0] -> chunk 0 col 0, hRL[s] -> chunk nch-1 col 127
            h0, w0 = cmap[0]
            nc.vector.tensor_copy(
                hcatP1[h0 * 64 : h0 * 64 + HP, w0 * 128 : w0 * 128 + bl],
                init_sb[0:HP, :],
            )
            h1, w1 = cmap[nch - 1]
            nc.vector.tensor_copy(
                hcatP1[h1 * 64 + HP : h1 * 64 + DH,
                       w1 * 128 + 128 - bl : w1 * 128 + 128],
                init_sb[HP:DH, :],
            )

            def lr_loc(i):
                """(rows, cols) of hLR[i] in hcatP1."""
                hh, ww = cmap[i // tw]
                return hh * 64, ww * 128 + (i % tw) * bl

            def rl_loc(i):
                """(rows, cols) of hRL[i+1] in hcatP1."""
                hh, ww = cmap[i // tw]
                return hh * 64 + HP, ww * 128 + (i % tw) * bl

            # ---- chunk-pipelined: transpose -> xproj -> prefill -> scan ----
            with (
                tc.tile_pool(name="pre_psum", bufs=2, space="PSUM") as prepsum,
                tc.tile_pool(name="xp_psum", bufs=2, space="PSUM") as xpp,
                tc.tile_pool(name="scanL", bufs=1, space="PSUM") as scL,
                tc.tile_pool(name="scanR", bufs=1, space="PSUM") as scR,
            ):
                pscanL = scL.tile([HP, VS], f32)
                pscanR = scR.tile([HP, VS], f32)
                for ch in range(nch):
                    cs = slice(ch * 128, (ch + 1) * 128)
                    for embg, ehi, elo in (
                        (embg_lr, embT_hi_lr, embT_lo_lr),
                        (embg_rl, embT_hi_rl, embT_lo_rl),
                    ):
                        tp = prepsum.tile([128, 128], f16, tag="tp")
                        nc.tensor.transpose(tp[:], embg[:, ch, 0:EH], ident)
                        nc.vector.tensor_copy(ehi[:, cs], tp[:])
                        tp2 = prepsum.tile([128, 128], f16, tag="tp")
                        nc.tensor.transpose(tp2[0:EL, :], embg[:, ch, EH:E], ident)
                        nc.vector.tensor_copy(elo[:, cs], tp2[0:EL, :])
                    for xp, whi, wlo, ehi, elo in (
                        (xpL, we_lr_hi, we_lr_lo, embT_hi_lr, embT_lo_lr),
                        (xpR, we_rl_hi, we_rl_lo, embT_hi_rl, embT_lo_rl),
                    ):
                        psx = xpp.tile([H, 128], f32, tag="xp")
                        nc.tensor.matmul(psx[:], whi, ehi[:, cs], start=True, stop=False)
                        nc.tensor.matmul(psx[:], wlo, elo[:, cs], start=False, stop=True)
                        nc.vector.tensor_copy(xp[0:H, cs], psx[:])
                    # prefill both chains' pre-activations (+bias, +8.0 lane)
                    pc0 = ch * 128
                    pcw = min(128, (s - 1) * bl - pc0)
                    if pcw > 0:
                        nc.tensor.matmul(
                            pscanL[:, pc0 : pc0 + pcw], iLb, xpL[:, pc0 : pc0 + pcw],
                            start=(ch == 0), stop=False, skip_group_check=True,
                        )
                        nc.tensor.matmul(
                            pscanR[:, pc0 : pc0 + pcw], iRb, xpR[:, pc0 : pc0 + pcw],
                            start=(ch == 0), stop=False, skip_group_check=True,
                        )
                    # scan steps whose xproj lives in this chunk
                    for t in range(ch * tw, min((ch + 1) * tw, s - 1)):
                        sl = slice(t * bl, (t + 1) * bl)
                        # L chain: hLR[t+1] = tanh(whL^T hLR[t] + xpL[t])
                        rr, rc = lr_loc(t)
                        nc.tensor.matmul(
                            pscanL[:, sl], whL[rr], hcatP1[rr : rr + HP, rc : rc + bl],
                            start=False, stop=(t == s - 2), skip_group_check=True,
                            tile_position=(rr, 0),
                        )
                        dr, dc = lr_loc(t + 1)
                        nc.scalar.activation(
                            hcatP1[dr : dr + HP, dc : dc + bl], pscanL[:, sl], Act.Tanh
                        )
                        # R chain: hRL[s-1-t] = tanh(whR^T hRL[s-t] + xpR_rev[t])
                        rr, rc = rl_loc(s - 1 - t)
                        nc.tensor.matmul(
                            pscanR[:, sl], whR[rr], hcatP1[rr : rr + HP, rc : rc + bl],
                            start=False, stop=(t == s - 2), skip_group_check=True,
                            tile_position=(rr, 0),
                        )
                        dr, dc = rl_loc(s - 2 - t)
                        nc.scalar.activation(
                            hcatP1[dr : dr + HP, dc : dc + bl], pscanR[:, sl], Act.Tanh
                        )

            # half-swapped copy: window w of hcatP2 rows 0:64 = hcatP1 rows 64:128
            for w_ in range(nwin):
                nc.vector.tensor_copy(
                    hcatP2[0:64, w_ * 128 : (w_ + 1) * 128],
                    hcatP1[64:128, w_ * 128 : (w_ + 1) * 128],
                )

            # ---- output projection + log_softmax ----------------------------
            # software-pipelined: pass2 of chunk m interleaves with pass1 of
            # chunk m+1 at supertile granularity so ACT (exp), DVE (sub) and
            # DMA (store) all stream concurrently.
            def lhs_of(ch):
                half, win = cmap[ch]
                t_ = hcatP1 if half == 0 else hcatP2
                return t_[:, win * 128 : (win + 1) * 128]

            def emit_p1_tile(lhs, sti, sums):
                v0, w = sup_tiles[sti]
                ps = op1.tile([128, SUP], f32, tag="ops1")
                for k0, kw in _splits512(w):
                    nc.tensor.matmul(
                        ps[:, k0 : k0 + kw], lhs,
                        w_dup[:, v0 + k0 : v0 + k0 + kw],
                        start=True, stop=True,
                    )
                esc = escp.tile([128, SUP], f32, tag="esc")
                nc.scalar.activation(
                    esc[:, 0:w], ps[:, 0:w], Act.Exp,
                    accum_out=sums[:, sti : sti + 1],
                )

            def emit_newton(sums):
                z = statp.tile([128, 1], f32, tag="z")
                nc.vector.tensor_reduce(
                    z[:], sums[:, 0:ns],
                    axis=mybir.AxisListType.X, op=mybir.AluOpType.add,
                )
                y = statp.tile([128, 1], f32, tag="y")
                nc.vector.tensor_scalar(
                    out=y[:], in0=z[:, 0:1].bitcast(i32),
                    scalar1=LN2 / (1 << 23), scalar2=-LN2 * 126.955,
                    op0=mybir.AluOpType.mult, op1=mybir.AluOpType.add,
                )
                for _ in range(3):
                    e = statp.tile([128, 1], f32, tag="e")
                    nc.scalar.activation(e[:], y[:], Act.Exp, scale=-1.0)
                    tmz = statp.tile([128, 1], f32, tag="t")
                    nc.vector.tensor_tensor(
                        out=tmz[:], in0=e[:], in1=z[:], op=mybir.AluOpType.mult
                    )
                    yn = statp.tile([128, 1], f32, tag="y")
                    nc.vector.tensor_tensor(
                        out=yn[:], in0=y[:], in1=tmz[:], op=mybir.AluOpType.add
                    )
                    y = yn
                    nc.vector.tensor_scalar_add(y[:], y[:], -1.0)
                return y

            with (
                tc.tile_pool(name="p1_psum", bufs=2, space="PSUM") as op1,
                tc.tile_pool(name="p2_psum", bufs=2, space="PSUM") as op2,
            ):
                state = {}  # per-chunk: sums, y, staging group
                sums0 = statp.tile([128, ns], f32, tag="sums0")
                for sti in range(ns):
                    emit_p1_tile(lhs_of(order[0]), sti, sums0)
                y_cur = emit_newton(sums0)
                for i, ch in enumerate(order):
                    lhs = lhs_of(ch)
                    nxt = order[i + 1] if i + 1 < nch else None
                    if nxt is not None:
                        sums_n = statp.tile([128, ns], f32, tag="sums1")
                    sg = None
                    for sti, (v0, w) in enumerate(sup_tiles):
                        # pass2 supertile of current chunk
                        ps = op2.tile([128, SUP], f32, tag="ops2")
                        for k0, kw in _splits512(w):
                            nc.tensor.matmul(
                                ps[:, k0 : k0 + kw], lhs,
                                w_dup[:, v0 + k0 : v0 + k0 + kw],
                                start=True, stop=True,
                            )
                        if sg is None:
                            stg = stp.tile([128, 4 * SUP], f32, tag="stg")
                            sg = (v0, stg)
                        g0, stg = sg
                        nc.vector.tensor_scalar_sub(
                            stg[:, v0 - g0 : v0 - g0 + w], ps[:, 0:w], y_cur[:, 0:1]
                        )
                        if sti == ns - 1 or v0 - g0 + w >= 4 * SUP:
                            gw = v0 - g0 + w
                            nc.sync.dma_start(
                                out_d[ch * 128 : (ch + 1) * 128, g0 : g0 + gw],
                                stg[:, 0:gw],
                            )
                            sg = None
                        # pass1 supertile of next chunk rides along
                        if nxt is not None:
                            emit_p1_tile(lhs_of(nxt), sti, sums_n)
                    if nxt is not None:
                        y_cur = emit_newton(sums_n)

    nc.compile()
    return nc


def prep_host_inputs(inputs, s=S, bl=BL, v=V, ncores=NCORES):
    """Slice/repack the full inputs into one in_map per core."""
    ib = np.asarray(inputs["input_batch"]).astype(np.int32)        # (s, B)
    emb = np.ascontiguousarray(np.asarray(inputs["embedding"], dtype=np.float32))
    W_lr = np.asarray(inputs["W_ih_lr"], dtype=np.float32)          # (E+H, H)
    b_lr = np.asarray(inputs["b_ih_lr"], dtype=np.float32)          # (1, H)
    W_rl = np.asarray(inputs["W_ih_rl"], dtype=np.float32)
    b_rl = np.asarray(inputs["b_ih_rl"], dtype=np.float32)
    W_ho = np.asarray(inputs["W_ho"], dtype=np.float32)             # (2H, v)
    b_ho = np.asarray(inputs["b_ho"], dtype=np.float32)             # (1, v)
    init = np.asarray(inputs["initial_hidden"], dtype=np.float32)   # (1, H)

    r = s * bl
    nch = r // 128
    c_init = C_INIT + bl

    w_dup = np.zeros((128, v), np.float16)
    w_dup[0:H] = W_ho[0:H].astype(np.float16)
    w_dup[HP : HP + H] = W_ho[H : 2 * H].astype(np.float16)
    w_dup[LANE] = b_ho[0].astype(np.float16)      # lane value is exactly 1.0

    s16 = np.zeros((128, c_init), np.float16)
    s16[:, C_WLRH : C_WLRH + H] = W_lr[:EH]
    s16[:, C_WRLH : C_WRLH + H] = W_rl[:EH]
    s16[0:EL, C_WLRL : C_WLRL + H] = W_lr[EH:E]
    s16[0:EL, C_WRLL : C_WRLL + H] = W_rl[EH:E]
    # scan weights, dup'd for both partition bases
    s16[0:H, C_WH : C_WH + H] = W_lr[E : E + H]
    s16[64 : 64 + H, C_WH : C_WH + H] = W_lr[E : E + H]
    s16[HP : HP + H, C_WH : C_WH + H] = W_rl[E : E + H]
    s16[96 : 96 + H, C_WH : C_WH + H] = W_rl[E : E + H]
    # identity-plus-bias prefill weights
    s16[0:HP, C_ILB : C_ILB + HP] = np.eye(HP, dtype=np.float16)
    s16[HP, C_ILB : C_ILB + H] = b_lr[0]
    s16[0:HP, C_IRB : C_IRB + HP] = np.eye(HP, dtype=np.float16)
    s16[HP, C_IRB : C_IRB + H] = b_rl[0]
    s16[HP, C_IRB + H] = 8.0                      # tanh(8) == 1.0 in fp16 (lane)
    s16[0:H, C_INIT : c_init] = init.T
    s16[HP : HP + H, C_INIT : c_init] = init.T
    s16[LANE, C_INIT : c_init] = 1.0              # lane state in init too

    s32 = np.zeros((128, 128), np.float16)
    s32[:, 0:128] = np.eye(128, dtype=np.float16)

    shared = {"emb": emb, "w_dup": w_dup, "smalls16": s16, "smalls32": s32}
    in_maps = []
    for c in range(ncores):
        ibc = ib[:, c * bl : (c + 1) * bl]                    # (s, bl)
        flat_lr = ibc.reshape(-1)                             # r = t*bl + b
        flat_rl = ibc[::-1].reshape(-1)
        idxp = np.empty((128, 2 * nch), np.int32)
        idxp[:, 0:nch] = flat_lr.reshape(nch, 128).T
        idxp[:, nch : 2 * nch] = flat_rl.reshape(nch, 128).T
        in_maps.append(dict(shared, idx=idxp))
    return in_maps


_CACHED = {}


def _get_program():
    if "nc" not in _CACHED:
        _CACHED["nc"] = build_program()
    return _CACHED["nc"]


def run_on_hw(inputs, trace=False):
    from concourse.bass_utils import run_bass_kernel_spmd

    nc = _get_program()
    in_maps = prep_host_inputs(inputs)
    res = run_bass_kernel_spmd(
        nc, in_maps, core_ids=list(range(NCORES)), trace=trace
    )
    out = np.empty((S, B, V), np.float32)
    for c in range(NCORES):
        out[:, c * BL : (c + 1) * BL, :] = res.results[c]["out"].reshape(S, BL, V)
    return out, res


def kernel(**inputs):
    out, _ = run_on_hw(inputs, trace=False)
    return out


# revision 15
# speedup vs baseline: 1.2620x; 1.0320x over previous
"""BiRNN language-model kernel for 8 Trainium2 NeuronCores.

Strategy: data-parallel over the batch dim (B=32 -> 4 per core), no
collectives.  Per core:
  1. indirect-DMA gather of the core's S*4 embedding rows (natural order
     for the L->R scan, time-reversed order for the R->L scan)
  2. per-128-token-chunk: PE transposes -> x-projection matmuls into
     xpL/xpR[33, S*4] (rows 0:30 = W_e^T emb per direction, row 32 =
     ones), pre-injected together with the input biases and a constant
     tanh(8)==1 lane (which later carries b_ho) into two PSUM banks.
  3. sequential scan as TWO independent chains (L->R and R->L), each
     step ONE accumulating [32,32]@[32,4] fp16 matmul + tanh that writes
     its hcat slice directly; the chains interleave on PE/ACT so the
     effective step cost is roughly halved vs a fused chain.
  4. output projection + log_softmax over V=32000 in two passes:
     pass1 logits -> PSUM, ACT exp with accum_out row sums; logZ = ln(Z)
     via an exponent-field estimate + 3 exp-Newton steps (exp only -- no
     ACT table switch, tanh/exp share one set).  pass2 recomputes logits
     and subtracts logZ during the PSUM->SBUF copy; 1MB DMA stores.

Hardware notes this shape exploits (measured here):
  - fp32 matmuls run 4 cycles/row (LOW_HIGH); fp16/bf16 run 1 cycle/row
    BUT only when the operands span 128 partitions -- K=64, N=512 fp16
    matmuls run at HALF rate.  Hence the output matmuls use K=128 with
    the top 64 weight rows zeroed, and the hidden states stored twice
    (hcatP1 and a half-swapped hcatP2) so each 128-row chunk's logits
    come from one full-partition matmul at 216ns.
  - SBUF access patterns must start at partition 0/32/64/96; direction
    blocks are padded 30->32 rows (zero weight rows kill the pads).
  - measured end-to-end rel err ~5e-5 (fp16 operands, fp32 accumulate).
"""

import sys

import numpy as np

for _p in ("/opt/trn_rl_repo", "/root/.axon_site/_ro/trn_rl_repo"):
    if _p not in sys.path:
        sys.path.insert(0, _p)

# problem constants
S, B, V, E, H = 128, 32, 32000, 150, 30
NCORES = 8
BL = B // NCORES          # batch rows per core
HP = 32                   # H padded to the 32-partition alignment
DH = 2 * HP               # 64: stacked direction state rows per chunk-half
LANE = 62                 # constant-one lane (carries b_ho): RL pad row 30
EH = 128                  # embedding dims handled by the "hi" K-split
EL = E - EH               # 22 remaining dims
VS = 512                  # fp32 matmul free-dim max (one PSUM bank)
SUP = 1024                # supertile: 2 PSUM banks per pool
LN2 = float(np.log(2.0))

# packed "smalls16" column layout (fp16, [128, n]):
#  whL dup'd at rows 0:32 & 64:96; whR dup'd at rows 32:64 & 96:128
C_WLRH, C_WRLH, C_WLRL, C_WRLL = 0, 30, 60, 90
C_WH, C_ILB, C_IRB, C_INIT = 120, 152, 184, 216
C_S16 = C_INIT + BL


def _v_supertiles(v_total):
    tiles = []
    v0 = 0
    while v0 < v_total:
        w = min(SUP, v_total - v0)
        tiles.append((v0, w))
        v0 += w
    return tiles


def _splits512(w):
    out = []
    k0 = 0
    while k0 < w:
        kw = min(VS, w - k0)
        out.append((k0, kw))
        k0 += kw
    return out


def _chunk_map(s, bl, nch):
    """chunk -> (half, window) of hcatP1, ordered by scan-readiness."""
    tw = 128 // bl
    ready = lambda ch: max(tw * ch + tw - 2, s - 2 - tw * ch)
    order = sorted(range(nch), key=ready)
    cmap = {ch: (pos % 2, pos // 2) for pos, ch in enumerate(order)}
    return cmap, order


def build_program(s=S, bl=BL, v=V):
    """Build the per-core Bass program (identical on all cores)."""
    from concourse import bacc, mybir
    import concourse.tile as tile

    f32 = mybir.dt.float32
    f16 = mybir.dt.float16
    i32 = mybir.dt.int32
    Act = mybir.ActivationFunctionType

    r = s * bl                 # rows per core
    nch = r // 128             # 128-row chunks
    tw = 128 // bl             # tokens per chunk
    assert r % 256 == 0, "need an even number of 128-row chunks"
    sup_tiles = _v_supertiles(v)
    ns = len(sup_tiles)
    cmap, order = _chunk_map(s, bl, nch)
    c_init = C_INIT + bl

    nc = bacc.Bacc(None, target_bir_lowering=False)

    idx_d = nc.dram_tensor("idx", [128, 2 * nch], i32, kind="ExternalInput")
    emb_d = nc.dram_tensor("emb", [V, E], f32, kind="ExternalInput")
    w_dup_d = nc.dram_tensor("w_dup", [128, v], f16, kind="ExternalInput")
    s16_d = nc.dram_tensor("smalls16", [128, c_init], f16, kind="ExternalInput")
    s32_d = nc.dram_tensor("smalls32", [128, 128], f16, kind="ExternalInput")
    out_d = nc.dram_tensor("out", [r, v], f32, kind="ExternalOutput")

    from concourse import bass

    with tile.TileContext(nc) as tc:
        with (
            tc.tile_pool(name="persist", bufs=1) as pp,
            tc.tile_pool(name="stage", bufs=3) as stp,
            tc.tile_pool(name="esc", bufs=2) as escp,
            tc.tile_pool(name="stat", bufs=4) as statp,
        ):
            # ---- input loads (idx first: the gather chain is the long pole)
            idx = pp.tile([128, 2 * nch], i32)
            nc.sync.dma_start(idx[:], idx_d[:])
            s16 = pp.tile([128, c_init], f16)
            nc.sync.dma_start(s16[:], s16_d[:])
            s32 = pp.tile([128, 128], f16)
            nc.sync.dma_start(s32[:], s32_d[:])
            w_dup = pp.tile([128, v], f16)
            nc.sync.dma_start(w_dup[:], w_dup_d[:])

            ident = s32[:, 0:128]
            we_lr_hi = s16[:, C_WLRH : C_WLRH + H]
            we_rl_hi = s16[:, C_WRLH : C_WRLH + H]
            we_lr_lo = s16[0:EL, C_WLRL : C_WLRL + H]
            we_rl_lo = s16[0:EL, C_WRLL : C_WRLL + H]
            whL = {0: s16[0:HP, C_WH : C_WH + HP], 64: s16[64:96, C_WH : C_WH + HP]}
            whR = {32: s16[HP:DH, C_WH : C_WH + HP], 96: s16[96:128, C_WH : C_WH + HP]}
            iLb = s16[0 : HP + 1, C_ILB : C_ILB + HP]
            iRb = s16[0 : HP + 1, C_IRB : C_IRB + HP]
            init_sb = s16[0:DH, C_INIT : C_INIT + bl]

            # ---- gathers (all issued up front; chunks stream through) -----
            embg_lr = pp.tile([128, nch, E], f16)
            embg_rl = pp.tile([128, nch, E], f16)
            for j in range(nch):
                nc.gpsimd.indirect_dma_start(
                    out=embg_lr[:, j, :], out_offset=None, in_=emb_d[:],
                    in_offset=bass.IndirectOffsetOnAxis(ap=idx[:, j : j + 1], axis=0),
                )
                nc.gpsimd.indirect_dma_start(
                    out=embg_rl[:, j, :], out_offset=None, in_=emb_d[:],
                    in_offset=bass.IndirectOffsetOnAxis(
                        ap=idx[:, nch + j : nch + j + 1], axis=0
                    ),
                )

            embT_hi_lr = pp.tile([EH, r], f16)
            embT_hi_rl = pp.tile([EH, r], f16)
            embT_lo_lr = pp.tile([EL, r], f16)
            embT_lo_rl = pp.tile([EL, r], f16)

            xpL = pp.tile([HP + 1, r], f16)      # row 32 = ones (bias inject)
            nc.vector.memset(xpL[:], 0.0)
            nc.vector.memset(xpL[HP : HP + 1, :], 1.0)
            xpR = pp.tile([HP + 1, r], f16)
            nc.vector.memset(xpR[:], 0.0)
            nc.vector.memset(xpR[HP : HP + 1, :], 1.0)

            nwin = nch // 2
            hcatP1 = pp.tile([128, nwin * 128], f16)
            nc.vector.memset(hcatP1[:], 0.0)
            hcatP2 = pp.tile([128, nwin * 128], f16)
            nc.vector.memset(hcatP2[64:128, :], 0.0)

            # init states: hLR[0] -> chunk 0 col 0, hRL[s] -> chunk nch-1 col 127
            h0, w0 = cmap[0]
            nc.vector.tensor_copy(
                hcatP1[h0 * 64 : h0 * 64 + HP, w0 * 128 : w0 * 128 + bl],
                init_sb[0:HP, :],
            )
            h1, w1 = cmap[nch - 1]
            nc.vector.tensor_copy(
                hcatP1[h1 * 64 + HP : h1 * 64 + DH,
                       w1 * 128 + 128 - bl : w1 * 128 + 128],
                init_sb[HP:DH, :],
            )

            def lr_loc(i):
                """(rows, cols) of hLR[i] in hcatP1."""
                hh, ww = cmap[i // tw]
                return hh * 64, ww * 128 + (i % tw) * bl

            def rl_loc(i):
                """(rows, cols) of hRL[i+1] in hcatP1."""
                hh, ww = cmap[i // tw]
                return hh * 64 + HP, ww * 128 + (i % tw) * bl

            # ---- chunk-pipelined: transpose -> xproj -> prefill -> scan ----
            with (
                tc.tile_pool(name="pre_psum", bufs=2, space="PSUM") as prepsum,
                tc.tile_pool(name="xp_psum", bufs=2, space="PSUM") as xpp,
                tc.tile_pool(name="scanL", bufs=1, space="PSUM") as scL,
                tc.tile_pool(name="scanR", bufs=1, space="PSUM") as scR,
            ):
                pscanL = scL.tile([HP, VS], f32)
                pscanR = scR.tile([HP, VS], f32)
                for ch in range(nch):
                    cs = slice(ch * 128, (ch + 1) * 128)
                    for embg, ehi, elo in (
                        (embg_lr, embT_hi_lr, embT_lo_lr),
                        (embg_rl, embT_hi_rl, embT_lo_rl),
                    ):
                        tp = prepsum.tile([128, 128], f16, tag="tp")
                        nc.tensor.transpose(tp[:], embg[:, ch, 0:EH], ident)
                        nc.vector.tensor_copy(ehi[:, cs], tp[:])
                        tp2 = prepsum.tile([128, 128], f16, tag="tp")
                        nc.tensor.transpose(tp2[0:EL, :], embg[:, ch, EH:E], ident)
                        nc.vector.tensor_copy(elo[:, cs], tp2[0:EL, :])
                    for xp, whi, wlo, ehi, elo in (
                        (xpL, we_lr_hi, we_lr_lo, embT_hi_lr, embT_lo_lr),
                        (xpR, we_rl_hi, we_rl_lo, embT_hi_rl, embT_lo_rl),
                    ):
                        psx = xpp.tile([H, 128], f32, tag="xp")
                        nc.tensor.matmul(psx[:], whi, ehi[:, cs], start=True, stop=False)
                        nc.tensor.matmul(psx[:], wlo, elo[:, cs], start=False, stop=True)
                        nc.vector.tensor_copy(xp[0:H, cs], psx[:])
                # prefill both chains' pre-activations (+bias, +8.0 lane)
                for ch in range(nch):
                    pc0 = ch * 128
                    pcw = min(128, (s - 1) * bl - pc0)
                    if pcw > 0:
                        nc.tensor.matmul(
                            pscanL[:, pc0 : pc0 + pcw], iLb, xpL[:, pc0 : pc0 + pcw],
                            start=(ch == 0), stop=False, skip_group_check=True,
                        )
                        nc.tensor.matmul(
                            pscanR[:, pc0 : pc0 + pcw], iRb, xpR[:, pc0 : pc0 + pcw],
                            start=(ch == 0), stop=False, skip_group_check=True,
                        )
                # the scan
                for t in range(s - 1):
                        sl = slice(t * bl, (t + 1) * bl)
                        # L chain: hLR[t+1] = tanh(whL^T hLR[t] + xpL[t])
                        rr, rc = lr_loc(t)
                        nc.tensor.matmul(
                            pscanL[:, sl], whL[rr], hcatP1[rr : rr + HP, rc : rc + bl],
                            start=False, stop=(t == s - 2), skip_group_check=True,
                            tile_position=(rr, 0),
                        )
                        dr, dc = lr_loc(t + 1)
                        nc.scalar.activation(
                            hcatP1[dr : dr + HP, dc : dc + bl], pscanL[:, sl], Act.Tanh
                        )
                        # R chain: hRL[s-1-t] = tanh(whR^T hRL[s-t] + xpR_rev[t])
                        rr, rc = rl_loc(s - 1 - t)
                        nc.tensor.matmul(
                            pscanR[:, sl], whR[rr], hcatP1[rr : rr + HP, rc : rc + bl],
                            start=False, stop=(t == s - 2), skip_group_check=True,
                            tile_position=(rr, 0),
                        )
                        dr, dc = rl_loc(s - 2 - t)
                        nc.scalar.activation(
                            hcatP1[dr : dr + HP, dc : dc + bl], pscanR[:, sl], Act.Tanh
                        )

            # half-swapped copy: window w of hcatP2 rows 0:64 = hcatP1 rows 64:128
            for w_ in range(nwin):
                nc.vector.tensor_copy(
                    hcatP2[0:64, w_ * 128 : (w_ + 1) * 128],
                    hcatP1[64:128, w_ * 128 : (w_ + 1) * 128],
                )

            # ---- output projection + log_softmax ----------------------------
            # software-pipelined: pass2 of chunk m interleaves with pass1 of
            # chunk m+1 at supertile granularity so ACT (exp), DVE (sub) and
            # DMA (store) all stream concurrently.
            def lhs_of(ch):
                half, win = cmap[ch]
                t_ = hcatP1 if half == 0 else hcatP2
                return t_[:, win * 128 : (win + 1) * 128]

            def emit_p1_tile(lhs, sti, sums):
                v0, w = sup_tiles[sti]
                ps = op1.tile([128, SUP], f32, tag="ops1")
                for k0, kw in _splits512(w):
                    nc.tensor.matmul(
                        ps[:, k0 : k0 + kw], lhs,
                        w_dup[:, v0 + k0 : v0 + k0 + kw],
                        start=True, stop=True,
                    )
                esc = escp.tile([128, SUP], f32, tag="esc")
                nc.scalar.activation(
                    esc[:, 0:w], ps[:, 0:w], Act.Exp,
                    accum_out=sums[:, sti : sti + 1],
                )

            def emit_newton(sums):
                z = statp.tile([128, 1], f32, tag="z")
                nc.vector.tensor_reduce(
                    z[:], sums[:, 0:ns],
                    axis=mybir.AxisListType.X, op=mybir.AluOpType.add,
                )
                y = statp.tile([128, 1], f32, tag="y")
                nc.vector.tensor_scalar(
                    out=y[:], in0=z[:, 0:1].bitcast(i32),
                    scalar1=LN2 / (1 << 23), scalar2=-LN2 * 126.955,
                    op0=mybir.AluOpType.mult, op1=mybir.AluOpType.add,
                )
                for _ in range(3):
                    e = statp.tile([128, 1], f32, tag="e")
                    nc.scalar.activation(e[:], y[:], Act.Exp, scale=-1.0)
                    tmz = statp.tile([128, 1], f32, tag="t")
                    nc.vector.tensor_tensor(
                        out=tmz[:], in0=e[:], in1=z[:], op=mybir.AluOpType.mult
                    )
                    yn = statp.tile([128, 1], f32, tag="y")
                    nc.vector.tensor_tensor(
                        out=yn[:], in0=y[:], in1=tmz[:], op=mybir.AluOpType.add
                    )
                    y = yn
                    nc.vector.tensor_scalar_add(y[:], y[:], -1.0)
                return y

            with (
                tc.tile_pool(name="p1_psum", bufs=2, space="PSUM") as op1,
                tc.tile_pool(name="p2_psum", bufs=2, space="PSUM") as op2,
            ):
                state = {}  # per-chunk: sums, y, staging group
                sums0 = statp.tile([128, ns], f32, tag="sums0")
                for sti in range(ns):
                    emit_p1_tile(lhs_of(order[0]), sti, sums0)
                y_cur = emit_newton(sums0)
                for i, ch in enumerate(order):
                    lhs = lhs_of(ch)
                    nxt = order[i + 1] if i + 1 < nch else None
                    if nxt is not None:
                        sums_n = statp.tile([128, ns], f32, tag="sums1")
                    ny = statp.tile([128, 1], f32, tag="ny")
                    nc.vector.tensor_scalar_mul(ny[:], y_cur[:, 0:1], -1.0)
                    sg = None
                    for sti, (v0, w) in enumerate(sup_tiles):
                        # pass2 supertile of current chunk
                        ps = op2.tile([128, SUP], f32, tag="ops2")
                        for k0, kw in _splits512(w):
                            nc.tensor.matmul(
                                ps[:, k0 : k0 + kw], lhs,
                                w_dup[:, v0 + k0 : v0 + k0 + kw],
                                start=True, stop=True,
                            )
                        if sg is None:
                            stg = stp.tile([128, 4 * SUP], f32, tag="stg")
                            sg = (v0, stg)
                        g0, stg = sg
                        if i >= nch - 2 and sti % 2 == 1:
                            # ACT idles at the tail -- give it half the copies
                            nc.scalar.add(
                                stg[:, v0 - g0 : v0 - g0 + w], ps[:, 0:w],
                                ny[:, 0:1],
                            )
                        else:
                            nc.vector.tensor_scalar_sub(
                                stg[:, v0 - g0 : v0 - g0 + w], ps[:, 0:w], y_cur[:, 0:1]
                            )
                        if sti == ns - 1 or v0 - g0 + w >= 4 * SUP:
                            gw = v0 - g0 + w
                            nc.sync.dma_start(
                                out_d[ch * 128 : (ch + 1) * 128, g0 : g0 + gw],
                                stg[:, 0:gw],
                            )
                            sg = None
                        # pass1 supertile of next chunk rides along
                        if nxt is not None:
                            emit_p1_tile(lhs_of(nxt), sti, sums_n)
                    if nxt is not None:
                        y_cur = emit_newton(sums_n)

    nc.compile()
    return nc


def prep_host_inputs(inputs, s=S, bl=BL, v=V, ncores=NCORES):
    """Slice/repack the full inputs into one in_map per core."""
    ib = np.asarray(inputs["input_batch"]).astype(np.int32)        # (s, B)
    emb = np.ascontiguousarray(np.asarray(inputs["embedding"], dtype=np.float32))
    W_lr = np.asarray(inputs["W_ih_lr"], dtype=np.float32)          # (E+H, H)
    b_lr = np.asarray(inputs["b_ih_lr"], dtype=np.float32)          # (1, H)
    W_rl = np.asarray(inputs["W_ih_rl"], dtype=np.float32)
    b_rl = np.asarray(inputs["b_ih_rl"], dtype=np.float32)
    W_ho = np.asarray(inputs["W_ho"], dtype=np.float32)             # (2H, v)
    b_ho = np.asarray(inputs["b_ho"], dtype=np.float32)             # (1, v)
    init = np.asarray(inputs["initial_hidden"], dtype=np.float32)   # (1, H)

    r = s * bl
    nch = r // 128
    c_init = C_INIT + bl

    w_dup = np.zeros((128, v), np.float16)
    w_dup[0:H] = W_ho[0:H].astype(np.float16)
    w_dup[HP : HP + H] = W_ho[H : 2 * H].astype(np.float16)
    w_dup[LANE] = b_ho[0].astype(np.float16)      # lane value is exactly 1.0

    s16 = np.zeros((128, c_init), np.float16)
    s16[:, C_WLRH : C_WLRH + H] = W_lr[:EH]
    s16[:, C_WRLH : C_WRLH + H] = W_rl[:EH]
    s16[0:EL, C_WLRL : C_WLRL + H] = W_lr[EH:E]
    s16[0:EL, C_WRLL : C_WRLL + H] = W_rl[EH:E]
    # scan weights, dup'd for both partition bases
    s16[0:H, C_WH : C_WH + H] = W_lr[E : E + H]
    s16[64 : 64 + H, C_WH : C_WH + H] = W_lr[E : E + H]
    s16[HP : HP + H, C_WH : C_WH + H] = W_rl[E : E + H]
    s16[96 : 96 + H, C_WH : C_WH + H] = W_rl[E : E + H]
    # identity-plus-bias prefill weights
    s16[0:HP, C_ILB : C_ILB + HP] = np.eye(HP, dtype=np.float16)
    s16[HP, C_ILB : C_ILB + H] = b_lr[0]
    s16[0:HP, C_IRB : C_IRB + HP] = np.eye(HP, dtype=np.float16)
    s16[HP, C_IRB : C_IRB + H] = b_rl[0]
    s16[HP, C_IRB + H] = 8.0                      # tanh(8) == 1.0 in fp16 (lane)
    s16[0:H, C_INIT : c_init] = init.T
    s16[HP : HP + H, C_INIT : c_init] = init.T
    s16[LANE, C_INIT : c_init] = 1.0              # lane state in init too

    s32 = np.zeros((128, 128), np.float16)
    s32[:, 0:128] = np.eye(128, dtype=np.float16)

    shared = {"emb": emb, "w_dup": w_dup, "smalls16": s16, "smalls32": s32}
    in_maps = []
    for c in range(ncores):
        ibc = ib[:, c * bl : (c + 1) * bl]                    # (s, bl)
        flat_lr = ibc.reshape(-1)                             # r = t*bl + b
        flat_rl = ibc[::-1].reshape(-1)
        idxp = np.empty((128, 2 * nch), np.int32)
        idxp[:, 0:nch] = flat_lr.reshape(nch, 128).T
        idxp[:, nch : 2 * nch] = flat_rl.reshape(nch, 128).T
        in_maps.append(dict(shared, idx=idxp))
    return in_maps


_CACHED = {}


def _get_program():
    if "nc" not in _CACHED:
        _CACHED["nc"] = build_program()
    return _CACHED["nc"]


def run_on_hw(inputs, trace=False):
    from concourse.bass_utils import run_bass_kernel_spmd

    nc = _get_program()
    in_maps = prep_host_inputs(inputs)
    res = run_bass_kernel_spmd(
        nc, in_maps, core_ids=list(range(NCORES)), trace=trace
    )
    out = np.empty((S, B, V), np.float32)
    for c in range(NCORES):
        out[:, c * BL : (c + 1) * BL, :] = res.results[c]["out"].reshape(S, BL, V)
    return out, res


def kernel(**inputs):
    out, _ = run_on_hw(inputs, trace=False)
    return out
